# revision 7
# baseline (speedup 1.0000x reference)
"""GCMC layer (gnn_message_passing) Bass kernel for 8 Trainium2 NeuronCores.

Strategy (dest-sharded, no collectives):
  out_dis[m, r, :] = ci_dis[m] * (S_dis[r][m] @ Wfc_r) + fc_b
  where S_dis[r][m] = sum_{edges e of rating r with dst=m} x_drug[src[e]]
        x_drug[n]   = cj_drug[n] * drug_feat[n]      (bf16 gather table)
        Wfc_r       = (sum_b att[r,b]*basis[b]) @ fc_w    [F, OUT]
  (and symmetrically for the reverse direction dis->drug)

  - Host sorts edges of each (direction, rating) by destination, shards
    destinations across 8 cores, groups them into dest tiles of 128, and
    lays out per-tile edge chunks of 128 padded to a static chunk count.
  - Launch 1: each core scales its 1/8 slice of node features by cj -> bf16.
    Host concatenates the slices into full gather tables.
  - Launch 2 (main): per dest tile: batched SWDGE gathers (InstDMAGatherAnt,
    up to 7x128 rows per instruction -- the num_idxs field tops out below
    1024) fetch all the tile's message rows; DVE builds the one-hot
    P = is_equal(dstloc, iota); TensorE accumulates ZT[f,d] += M.T @ P in
    PSUM (the segment sum), a second matmul ZT.T @ Wfc_r lands in a
    per-tile [128, R*OUT] PSUM bank, DVE applies ci scale + bias, and the
    result is stored contiguously in the final [node, r, out] layout.
  - dma_gather indices are signed int16 (< 32768), so each gather reads
    through one of two overlapping 32768-row windows of the x table
    (bases 0 and TBL-32768).  Edges are assigned to a window by source id:
    src < 17280 must use window 0, src >= 32768 must use window 1, and the
    overlap is used as slack to balance the two static chunk budgets.
    Pad slots point at a harmless valid row; their dstloc is 200 so the
    one-hot match never fires and they contribute nothing.
"""

import json
import os
import time

import numpy as np
import ml_dtypes

_VERBOSE = os.environ.get("KERNEL_VERBOSE", "0") == "1"


def _tlog(msg, t0=[None]):
    if _VERBOSE:
        now = time.time()
        dt = 0.0 if t0[0] is None else now - t0[0]
        t0[0] = now
        print(f"[kernel +{dt:6.2f}s] {msg}", flush=True)

import concourse.bass as bass
import concourse.mybir as mybir
import concourse.tile as tile
from concourse import library_config
from concourse.bass_utils import run_bass_kernel_spmd

BF16 = ml_dtypes.bfloat16


# ----------------------------------------------------------------------
# Workaround: the staged walrus rejects >1 sync wait per instruction
# ("Too many sync wait commands") while the Tile scheduler emits multi-wait
# instructions.  Split extra waits into standalone EventSemaphore
# instructions right before the owning instruction (same engine queue, so
# semantics are identical: all waits are pre-conditions).
# ----------------------------------------------------------------------

def _split_multiwaits(bir: bytes) -> bytes:
    j = json.loads(bir)
    for fn in j["functions"]:
        for blk in fn["blocks"]:
            out = []
            k = 0
            for ins in blk["instructions"]:
                si = ins.get("sync_info") or {}
                waits = si.get("on_wait") or []
                if len(waits) > 1:
                    for w in waits[:-1]:
                        out.append({
                            "debug": ins.get("debug"),
                            "engine": ins["engine"],
                            "ins": [], "outs": [],
                            "name": f"{ins['name']}-ws{k}",
                            "opcode": "EventSemaphore",
                            "sync_info": {"on_update": [], "on_wait": [w]},
                        })
                        k += 1
                    si["on_wait"] = [waits[-1]]
                out.append(ins)
            blk["instructions"] = out
    return json.dumps(j).encode()


_orig_to_json_bytes = bass.Bass.to_json_bytes


def _patched_to_json_bytes(self):
    return _split_multiwaits(_orig_to_json_bytes(self))


bass.Bass.to_json_bytes = _patched_to_json_bytes

# ----- problem constants (hardcoded per contract) -----
N = 50000          # nodes per side
F = 128            # feature dim
R = 5              # ratings
E = 400000         # edges per rating per direction
OUT = 64           # output dim
NB = 2             # basis count
NCORES = 8

WIN = 32768        # dma_gather index window (signed int16)
GCAP = 7           # max 128-chunks per dma_gather (num_idxs < 1024)

f32 = mybir.dt.float32
bf16 = mybir.dt.bfloat16
i32 = mybir.dt.int32
i16 = mybir.dt.int16


def _derived():
    npc = N // NCORES
    nt = (npc + 127) // 128
    npad = nt * 128
    tbl = ((N + 1 + 127) // 128) * 128  # >= N+1 so row N exists and is zero
    return npc, nt, npad, tbl


def _splits(nchunks):
    """Split nchunks into groups of <= GCAP chunks: [(offset, count), ...]."""
    out = []
    o = 0
    while o < nchunks:
        g = min(GCAP, nchunks - o)
        out.append((o, g))
        o += g
    return out


# ======================================================================
# Host-side edge preprocessing
# ======================================================================

def _prep_direction(src_all, dst_all, C0, C1):
    """For one direction, build per-core gather-index streams + dstloc.

    Edges of each (rating, dest-tile) are split between two table windows
    (bases 0 and W1=TBL-WIN) by source id, padded to static chunk counts
    C0/C1 per rating.  If C0 is None, only computes the needed maxima
    (returns (maxC0, maxC1, maxC)).

    Returns (idx0, idx1, dstloc) with
      idx0: int16 [NCORES, 128, NT*R*C0*8]   window-0 gather indices (wrap16)
      idx1: int16 [NCORES, 128, NT*R*C1*8]
      dstloc: bf16 [NCORES, 128, NT*R*(C0+C1)]  dest offset / 200 for pads
    m-tile column layout per tile t: [r-major (r, j<C0) | r-major (r, j<C1)].
    """
    NPC, NT, _, TBLR = _derived()
    W1 = TBLR - WIN
    measure = C0 is None
    maxn0 = maxn1 = maxtot = 1
    if not measure:
        C0T, C1T = R * C0, R * C1
        CT = C0T + C1T
        idx0 = np.zeros((NCORES, NT * C0T * 128), np.int32)
        idx1 = np.zeros((NCORES, NT * C1T * 128), np.int32)
        dstloc = np.full((NCORES, NT * CT, 128), 200.0, np.float32)
    for r in range(R):
        order = np.argsort(dst_all[r], kind="stable")
        dst_s = dst_all[r][order].astype(np.int64)
        src_s = src_all[r][order].astype(np.int64)
        bounds = np.searchsorted(dst_s, np.arange(NCORES + 1) * NPC)
        for c in range(NCORES):
            lo, hi = bounds[c], bounds[c + 1]
            d = dst_s[lo:hi] - c * NPC
            s = src_s[lo:hi]
            t_id = d >> 7
            # category: 0 = must-window0 (s<W1), 1 = either, 2 = must-window1
            # final sort key includes src so each chunk's gather descriptors
            # walk the table monotonically (DRAM row locality)
            cat = np.where(s < W1, 0, np.where(s < WIN, 1, 2))
            key = t_id * 3 + cat
            o2 = np.argsort(key << 17 | s, kind="stable")
            d, s, t_id, cat, key = d[o2], s[o2], t_id[o2], cat[o2], key[o2]
            cnt = np.bincount(key, minlength=NT * 3).reshape(NT, 3)
            n0m, nov, n1m = cnt[:, 0], cnt[:, 1], cnt[:, 2]
            tot = n0m + nov + n1m
            maxn0 = max(maxn0, int(n0m.max()))
            maxn1 = max(maxn1, int(n1m.max()))
            maxtot = max(maxtot, int(tot.max()))
            if measure:
                continue
            # assign overlap edges: fill window-0 only as much as needed to
            # keep window-1 within its budget, and never beyond C0*128
            need = np.clip(tot - C1 * 128 - n0m, 0, None)
            a = np.clip(need, 0, np.minimum(nov, C0 * 128 - n0m))
            h0_sz = n0m + a
            assert (h0_sz <= C0 * 128).all() and (tot - h0_sz <= C1 * 128).all()
            t_start = np.zeros(NT + 1, np.int64)
            np.cumsum(tot, out=t_start[1:])
            pos = np.arange(len(d)) - t_start[t_id]
            in_h0 = pos < h0_sz[t_id]
            # window 0
            rank = pos[in_h0]
            tt = t_id[in_h0]
            j = rank >> 7
            cc = r * C0 + j
            q = (tt * C0T + cc) * 128 + (rank & 127)
            idx0[c][q] = s[in_h0]
            dstloc[c][tt * CT + cc, rank & 127] = d[in_h0] & 127
            # window 1
            rank = (pos - h0_sz[t_id])[~in_h0]
            tt = t_id[~in_h0]
            j = rank >> 7
            cc = r * C1 + j
            q = (tt * C1T + cc) * 128 + (rank & 127)
            idx1[c][q] = s[~in_h0] - W1
            dstloc[c][tt * CT + C0T + cc, rank & 127] = d[~in_h0] & 127
    if measure:
        return maxn0, maxn1, maxtot

    def wrap16(a):
        # flat [cores, L] -> [cores, 128, L//16] (wrapped, replicated 8x)
        L = a.shape[1]
        w = a.reshape(NCORES, L // 16, 16).transpose(0, 2, 1)  # [c, 16, L/16]
        return np.ascontiguousarray(
            np.broadcast_to(w[:, None, :, :], (NCORES, 8, 16, L // 16))
            .reshape(NCORES, 128, L // 16)).astype(np.int16)

    dstloc = np.ascontiguousarray(
        dstloc.transpose(0, 2, 1)).astype(BF16)  # [c, 128, NT*CT]
    return wrap16(idx0), wrap16(idx1), dstloc


# ======================================================================
# Launch 1: build bf16 gather tables (x = cj * feat), row-sharded
# ======================================================================

def build_prep_nc():
    NPC, NT, NPAD, TBL = _derived()
    nc = bass.Bass()
    feat_in = nc.dram_tensor("feat_slice", (2, NPAD, F), f32, kind="ExternalInput")
    # cj host-transposed to [2, 128, NT]: element [s, i, t] = cj[s, t*128+i]
    cj_in = nc.dram_tensor("cj_slice", (2, 128, NT), f32, kind="ExternalInput")
    x_out = nc.dram_tensor("x_slice", (2, NPAD, F), bf16, kind="ExternalOutput")

    with tile.TileContext(nc) as tc:
        with (
            tc.tile_pool(name="cj", bufs=1) as cjp,
            tc.tile_pool(name="sb", bufs=6) as sb,
        ):
            cj_sb = cjp.tile([128, 2 * NT], f32, tag="cj")
            nc.sync.dma_start(
                out=cj_sb[:].rearrange("p (s t) -> p s t", s=2),
                in_=cj_in[:, :, :].rearrange("s p t -> p s t"))
            # absorber: advance DVE's clock past the cj DMA so later consumers
            # need only one wait (walrus allows a single sync wait per compute
            # instruction and the scheduler doesn't always split).
            scratch = cjp.tile([128, 1], f32, tag="scratch")
            nc.vector.tensor_copy(out=scratch[:], in_=cj_sb[:, :1])
            for side in range(2):
                for t in range(NT):
                    rows = slice(t * 128, (t + 1) * 128)
                    ft = sb.tile([128, F], f32, tag="ft")
                    nc.sync.dma_start(out=ft[:], in_=feat_in[side, rows, :])
                    xt = sb.tile([128, F], bf16, tag="xt")
                    c0 = side * NT + t
                    nc.vector.tensor_tensor(
                        out=xt[:], in0=ft[:],
                        in1=cj_sb[:, c0:c0 + 1].to_broadcast([128, F]),
                        op=mybir.AluOpType.mult,
                    )
                    nc.sync.dma_start(out=x_out[side, rows, :], in_=xt[:])
    return nc


# ======================================================================
# Launch 2: main kernel
# ======================================================================

def build_main_nc(C0: int, C1: int):
    NPC, NT, NPAD, TBL = _derived()
    W1 = TBL - WIN
    C0T, C1T = R * C0, R * C1
    CT = C0T + C1T
    nc = bass.Bass()
    x_drug = nc.dram_tensor("x_drug", (TBL, F), bf16, kind="ExternalInput")
    x_dis = nc.dram_tensor("x_dis", (TBL, F), bf16, kind="ExternalInput")
    idx0_in = nc.dram_tensor("idx0", (2, 128, NT * C0T * 8), i16, kind="ExternalInput")
    idx1_in = nc.dram_tensor("idx1", (2, 128, NT * C1T * 8), i16, kind="ExternalInput")
    dstloc_in = nc.dram_tensor("dstloc", (2, 128, NT * CT), bf16, kind="ExternalInput")
    # ci host-transposed to [2, 128, NT]
    ci_in = nc.dram_tensor("ci_pad", (2, 128, NT), f32, kind="ExternalInput")
    att_in = nc.dram_tensor("att", (R, NB), f32, kind="ExternalInput")
    basis_in = nc.dram_tensor("basis", (NB, F, F), f32, kind="ExternalInput")
    fcw_in = nc.dram_tensor("fc_w", (F, OUT), f32, kind="ExternalInput")
    fcb_in = nc.dram_tensor("fc_b", (OUT,), f32, kind="ExternalInput")
    out = nc.dram_tensor("out_part", (2, NPAD, R, OUT), f32, kind="ExternalOutput")

    iota_np = np.broadcast_to(np.arange(128, dtype=np.float32), (128, 128))
    iota_c = nc.inline_tensor(np.ascontiguousarray(iota_np), "iota_c")
    ident_c = nc.inline_tensor(np.eye(128, dtype=np.float32), "ident_c")
    ones_c = nc.inline_tensor(np.ones((1, 128), dtype=np.float32), "ones_c")

    eq = mybir.AluOpType.is_equal
    mult = mybir.AluOpType.mult
    add = mybir.AluOpType.add

    splits0 = _splits(C0T)
    splits1 = _splits(C1T)

    with tile.TileContext(nc) as tc:
        with (
            tc.tile_pool(name="const", bufs=1) as cp,
            tc.tile_pool(name="sb", bufs=4) as sb,
            tc.tile_pool(name="mp", bufs=3) as mp,
            tc.tile_pool(name="pp", bufs=3) as pp,
            tc.tile_pool(name="zp", bufs=4) as zp,
            tc.tile_pool(name="idxp", bufs=1) as idxp,
            tc.tile_pool(name="ps", bufs=4, space="PSUM") as ps,
            tc.tile_pool(name="ps2", bufs=4, space="PSUM") as ps2,
        ):
            nc.gpsimd.load_library(library_config.mlp)
            # one Pool register per distinct gather size (to_reg per call
            # would exhaust the register file across ~700 gathers)
            ni_regs = {gn: nc.gpsimd.to_reg(gn * 128)
                       for (_, gn) in set(splits0) | set(splits1)}
            # ---------- consts ----------
            iota_f = cp.tile([128, 128], f32, tag="iotaf")
            nc.sync.dma_start(out=iota_f[:], in_=iota_c[:, :])
            iota_t = cp.tile([128, 128], bf16, tag="iota")
            nc.vector.tensor_copy(out=iota_t[:], in_=iota_f[:])
            ident_t = cp.tile([128, 128], f32, tag="ident")
            nc.sync.dma_start(out=ident_t[:], in_=ident_c[:, :])
            ones_f32 = cp.tile([1, 128], f32, tag="ones32")
            nc.sync.dma_start(out=ones_f32[:], in_=ones_c[:, :])
            fcw_t = cp.tile([128, OUT], f32, tag="fcw")
            nc.sync.dma_start(out=fcw_t[:], in_=fcw_in[:, :])
            fcb_row = cp.tile([1, OUT], f32, tag="fcbrow")
            nc.sync.dma_start(out=fcb_row[:], in_=fcb_in[None, :])
            att_row = cp.tile([1, R * NB], f32, tag="attrow")
            nc.sync.dma_start(out=att_row[:],
                              in_=att_in[:, :].rearrange("r b -> () (r b)"))

            # ---------- W prep ----------
            attb_ps = ps2.tile([128, R * NB], f32, tag="o2")
            nc.tensor.matmul(out=attb_ps[:], lhsT=ones_f32[:], rhs=att_row[:],
                             start=True, stop=True)
            att_b = cp.tile([128, R * NB], f32, tag="attb")
            nc.vector.tensor_copy(out=att_b[:], in_=attb_ps[:])

            biasb_ps = ps2.tile([128, OUT], f32, tag="o2")
            nc.tensor.matmul(out=biasb_ps[:], lhsT=ones_f32[:], rhs=fcb_row[:],
                             start=True, stop=True)
            bias5 = cp.tile([128, R * OUT], f32, tag="bias5")
            for r in range(R):
                nc.vector.tensor_copy(out=bias5[:, r * OUT:(r + 1) * OUT],
                                      in_=biasb_ps[:])

            # basis[b] transposed: [e, f]
            bT = []
            for b in range(NB):
                bt_in = sb.tile([128, 128], f32, tag="bload")
                nc.sync.dma_start(out=bt_in[:], in_=basis_in[b, :, :])
                bt_ps = ps.tile([128, 128], f32, tag="zt")
                nc.tensor.transpose(out=bt_ps[:], in_=bt_in[:], identity=ident_t[:])
                bt_sb = cp.tile([128, 128], f32, tag=f"bT{b}")
                nc.vector.tensor_copy(out=bt_sb[:], in_=bt_ps[:])
                bT.append(bt_sb)

            wfc = cp.tile([128, R * OUT], bf16, tag="wfc")
            for r in range(R):
                wrt = sb.tile([128, 128], f32, tag="wrt")
                tmp = sb.tile([128, 128], f32, tag="wtmp")
                nc.vector.tensor_tensor(
                    out=tmp[:], in0=bT[1][:],
                    in1=att_b[:, 2 * r + 1:2 * r + 2].to_broadcast([128, 128]),
                    op=mult,
                )
                nc.vector.tensor_tensor(
                    out=wrt[:], in0=bT[0][:],
                    in1=att_b[:, 2 * r:2 * r + 1].to_broadcast([128, 128]),
                    op=mult,
                )
                nc.vector.tensor_tensor(out=wrt[:], in0=wrt[:], in1=tmp[:], op=add)
                wfc_ps = ps2.tile([128, OUT], f32, tag="o2")
                nc.tensor.matmul(out=wfc_ps[:], lhsT=wrt[:], rhs=fcw_t[:],
                                 start=True, stop=True)
                nc.scalar.copy(out=wfc[:, r * OUT:(r + 1) * OUT], in_=wfc_ps[:])

            # ---------- main loops ----------
            ci_sb = cp.tile([128, 2 * NT], f32, tag="ci")
            nc.sync.dma_start(
                out=ci_sb[:].rearrange("p (s t) -> p s t", s=2),
                in_=ci_in[:, :, :].rearrange("s p t -> p s t"))
            for d in range(2):
                x_src = x_dis if d else x_drug
                idx0_t = idxp.tile([128, NT * C0T * 8], i16, tag="idx0")
                nc.sync.dma_start(out=idx0_t[:], in_=idx0_in[d, :, :])
                idx1_t = idxp.tile([128, NT * C1T * 8], i16, tag="idx1")
                nc.sync.dma_start(out=idx1_t[:], in_=idx1_in[d, :, :])
                dl_t = idxp.tile([128, NT * CT], bf16, tag="dl")
                nc.sync.dma_start(out=dl_t[:], in_=dstloc_in[d, :, :])
                for t in range(NT):
                    rows = slice(t * 128, (t + 1) * 128)
                    ci_col = d * NT + t
                    m_t = mp.tile([128, CT * 128], bf16, tag="m")
                    for (go, gn) in splits0:
                        ni = gn * 128
                        nc.gpsimd.dma_gather(
                            m_t[:, go * 128:(go + gn) * 128].rearrange(
                                "p (c w) -> p c w", w=F),
                            x_src[0:WIN, :],
                            idx0_t[:, (t * C0T + go) * 8:(t * C0T + go + gn) * 8],
                            ni, ni_regs[gn], F,
                        )
                    for (go, gn) in splits1:
                        ni = gn * 128
                        nc.gpsimd.dma_gather(
                            m_t[:, (C0T + go) * 128:(C0T + go + gn) * 128]
                            .rearrange("p (c w) -> p c w", w=F),
                            x_src[W1:W1 + WIN, :],
                            idx1_t[:, (t * C1T + go) * 8:(t * C1T + go + gn) * 8],
                            ni, ni_regs[gn], F,
                        )
                    p_t = pp.tile([128, CT * 128], bf16, tag="p")
                    nc.vector.tensor_tensor(
                        out=p_t[:].rearrange("p (c f) -> p c f", c=CT),
                        in0=dl_t[:, t * CT:(t + 1) * CT][:, :, None].to_broadcast(
                            [128, CT, 128]),
                        in1=iota_t[:, None, :].to_broadcast([128, CT, 128]),
                        op=eq,
                    )
                    o2 = ps2.tile([128, R * OUT], f32, tag="o2")
                    for r in range(R):
                        cols = ([r * C0 + j for j in range(C0)] +
                                [C0T + r * C1 + j for j in range(C1)])
                        zt = ps.tile([128, 128], f32, tag="zt")
                        for k, cc in enumerate(cols):
                            nc.tensor.matmul(
                                out=zt[:],
                                lhsT=m_t[:, cc * 128:(cc + 1) * 128],
                                rhs=p_t[:, cc * 128:(cc + 1) * 128],
                                start=(k == 0), stop=(k == len(cols) - 1),
                            )
                        zt_sb = zp.tile([128, 128], bf16, tag="ztsb")
                        nc.scalar.copy(out=zt_sb[:], in_=zt[:])
                        nc.tensor.matmul(
                            out=o2[:, r * OUT:(r + 1) * OUT], lhsT=zt_sb[:],
                            rhs=wfc[:, r * OUT:(r + 1) * OUT],
                            start=True, stop=True,
                        )
                    ob = sb.tile([128, R * OUT], f32, tag="ob")
                    nc.vector.tensor_tensor(
                        out=ob[:], in0=o2[:],
                        in1=ci_sb[:, ci_col:ci_col + 1].to_broadcast(
                            [128, R * OUT]),
                        op=mult,
                    )
                    nc.vector.tensor_tensor(
                        out=ob[:], in0=ob[:], in1=bias5[:], op=add)
                    nc.sync.dma_start(
                        out=out[d, rows, :, :].rearrange("p r o -> p (r o)"),
                        in_=ob[:],
                    )
    mybir.codegen_inst_isa_subclasses(nc)
    return nc


# ======================================================================
# kernel entry
# ======================================================================

_cache: dict = {}


def kernel(drug_feat, dis_feat, cj_drug, ci_drug, cj_dis, ci_dis,
           att, basis, fc_w, fc_b, edge_drug, edge_dis):
    NPC, NT, NPAD, TBL = _derived()
    drug_feat = np.asarray(drug_feat, np.float32)
    dis_feat = np.asarray(dis_feat, np.float32)
    cj_drug = np.asarray(cj_drug, np.float32)
    ci_drug = np.asarray(ci_drug, np.float32)
    cj_dis = np.asarray(cj_dis, np.float32)
    ci_dis = np.asarray(ci_dis, np.float32)
    att = np.asarray(att, np.float32)
    basis = np.asarray(basis, np.float32)
    fc_w = np.asarray(fc_w, np.float32)
    fc_b = np.asarray(fc_b, np.float32)
    edge_drug = np.asarray(edge_drug, np.int32)
    edge_dis = np.asarray(edge_dis, np.int32)

    # ---- host preprocessing: edge sort/shard (index manipulation only) ----
    # direction 0: drug -> dis (dest = dis), direction 1: dis -> drug
    _tlog("start")
    n0a, n1a, ta = _prep_direction(edge_drug, edge_dis, None, None)
    n0b, n1b, tb = _prep_direction(edge_dis, edge_drug, None, None)
    C0 = (max(n0a, n0b) + 127) // 128
    Cm = (max(ta, tb) + 127) // 128
    C1 = max((max(n1a, n1b) + 127) // 128, Cm - C0)
    idx0_d0, idx1_d0, dl_d0 = _prep_direction(edge_drug, edge_dis, C0, C1)
    idx0_d1, idx1_d1, dl_d1 = _prep_direction(edge_dis, edge_drug, C0, C1)
    _tlog(f"host prep done C0={C0} C1={C1}")

    # ---- launch 1: build gather tables ----
    if "prep" not in _cache:
        _cache["prep"] = build_prep_nc()
    nc1 = _cache["prep"]

    in_maps1 = []
    for c in range(NCORES):
        rows = slice(c * NPC, (c + 1) * NPC)
        feat_slice = np.zeros((2, NPAD, F), np.float32)
        feat_slice[0, :NPC] = drug_feat[rows]
        feat_slice[1, :NPC] = dis_feat[rows]
        cj_slice = np.zeros((2, NPAD), np.float32)
        cj_slice[0, :NPC] = cj_drug[rows]
        cj_slice[1, :NPC] = cj_dis[rows]
        cj_slice = np.ascontiguousarray(
            cj_slice.reshape(2, NT, 128).transpose(0, 2, 1))
        in_maps1.append({"feat_slice": feat_slice, "cj_slice": cj_slice})
    _tlog("launch1 inputs built")
    res1 = run_bass_kernel_spmd(nc1, in_maps1, core_ids=list(range(NCORES)))
    _tlog("launch1 done")
    xs = [r["x_slice"] for r in res1.results]
    x_drug_full = np.zeros((TBL, F), BF16)
    x_dis_full = np.zeros((TBL, F), BF16)
    for c in range(NCORES):
        rows = slice(c * NPC, (c + 1) * NPC)
        x_drug_full[rows] = xs[c][0, :NPC]
        x_dis_full[rows] = xs[c][1, :NPC]

    # ---- launch 2: main ----
    key = ("main", C0, C1)
    if key not in _cache:
        _cache[key] = build_main_nc(C0, C1)
    nc2 = _cache[key]

    in_maps2 = []
    for c in range(NCORES):
        rows = slice(c * NPC, (c + 1) * NPC)
        ci_pad = np.zeros((2, NPAD), np.float32)
        ci_pad[0, :NPC] = ci_dis[rows]    # dir 0 dest = dis
        ci_pad[1, :NPC] = ci_drug[rows]   # dir 1 dest = drug
        ci_pad = np.ascontiguousarray(
            ci_pad.reshape(2, NT, 128).transpose(0, 2, 1))
        in_maps2.append({
            "x_drug": x_drug_full, "x_dis": x_dis_full,
            "idx0": np.stack([idx0_d0[c], idx0_d1[c]], axis=0),
            "idx1": np.stack([idx1_d0[c], idx1_d1[c]], axis=0),
            "dstloc": np.stack([dl_d0[c], dl_d1[c]], axis=0),
            "ci_pad": ci_pad,
            "att": att, "basis": basis, "fc_w": fc_w, "fc_b": fc_b,
        })
    _tlog("launch2 inputs built")
    res2 = run_bass_kernel_spmd(nc2, in_maps2, core_ids=list(range(NCORES)))
    _tlog("launch2 done")

    out_dis = np.concatenate(
        [r["out_part"][0, :NPC] for r in res2.results], axis=0)
    out_drug = np.concatenate(
        [r["out_part"][1, :NPC] for r in res2.results], axis=0)
    _tlog("assembled")
    return out_drug.astype(np.float32), out_dis.astype(np.float32)


# revision 13
# speedup vs baseline: 3.2446x; 3.2446x over previous
"""GCMC layer (gnn_message_passing) Bass kernel for 8 Trainium2 NeuronCores.

Strategy (dest-sharded, no collectives):
  out_dis[m, r, :] = ci_dis[m] * (S_dis[r][m] @ Wfc_r) + fc_b
  where S_dis[r][m] = sum_{edges e of rating r with dst=m} x_drug[src[e]]
        x_drug[n]   = cj_drug[n] * drug_feat[n]      (bf16 gather table)
        Wfc_r       = (sum_b att[r,b]*basis[b]) @ fc_w    [F, OUT]
  (and symmetrically for the reverse direction dis->drug)

  - Host sorts edges of each (direction, rating) by destination, shards
    destinations across 8 cores, groups them into dest tiles of 128, and
    lays out per-tile edge chunks of 128 padded to a static chunk count.
  - Launch 1: each core scales its 1/8 slice of node features by cj -> bf16.
    Host concatenates the slices into full gather tables.
  - Launch 2 (main): per dest tile: batched SWDGE gathers (InstDMAGatherAnt,
    up to 7x128 rows per instruction -- the num_idxs field tops out below
    1024) fetch all the tile's message rows; DVE builds the one-hot
    P = is_equal(dstloc, iota); TensorE accumulates ZT[f,d] += M.T @ P in
    PSUM (the segment sum), a second matmul ZT.T @ Wfc_r lands in a
    per-tile [128, R*OUT] PSUM bank, DVE applies ci scale + bias, and the
    result is stored contiguously in the final [node, r, out] layout.
  - dma_gather indices are signed int16 (< 32768), so each gather reads
    through one of two overlapping 32768-row windows of the x table
    (bases 0 and TBL-32768).  Edges are assigned to a window by source id:
    src < 17280 must use window 0, src >= 32768 must use window 1, and the
    overlap is used as slack to balance the two static chunk budgets.
    Pad slots point at a harmless valid row; their dstloc is 200 so the
    one-hot match never fires and they contribute nothing.
"""

import json
import os
import time

import numpy as np
import ml_dtypes

_VERBOSE = os.environ.get("KERNEL_VERBOSE", "0") == "1"


def _tlog(msg, t0=[None]):
    if _VERBOSE:
        now = time.time()
        dt = 0.0 if t0[0] is None else now - t0[0]
        t0[0] = now
        print(f"[kernel +{dt:6.2f}s] {msg}", flush=True)

import concourse.bass as bass
import concourse.mybir as mybir
import concourse.tile as tile
from concourse import library_config
from concourse.bass_utils import run_bass_kernel_spmd

BF16 = ml_dtypes.bfloat16


# ----------------------------------------------------------------------
# Workaround: the staged walrus rejects >1 sync wait per instruction
# ("Too many sync wait commands") while the Tile scheduler emits multi-wait
# instructions.  Split extra waits into standalone EventSemaphore
# instructions right before the owning instruction (same engine queue, so
# semantics are identical: all waits are pre-conditions).
# ----------------------------------------------------------------------

def _split_multiwaits(bir: bytes) -> bytes:
    j = json.loads(bir)
    for fn in j["functions"]:
        for blk in fn["blocks"]:
            out = []
            k = 0
            for ins in blk["instructions"]:
                si = ins.get("sync_info") or {}
                waits = si.get("on_wait") or []
                if len(waits) > 1:
                    for w in waits[:-1]:
                        out.append({
                            "debug": ins.get("debug"),
                            "engine": ins["engine"],
                            "ins": [], "outs": [],
                            "name": f"{ins['name']}-ws{k}",
                            "opcode": "EventSemaphore",
                            "sync_info": {"on_update": [], "on_wait": [w]},
                        })
                        k += 1
                    si["on_wait"] = [waits[-1]]
                out.append(ins)
            blk["instructions"] = out
    return json.dumps(j).encode()


_orig_to_json_bytes = bass.Bass.to_json_bytes


def _patched_to_json_bytes(self):
    return _split_multiwaits(_orig_to_json_bytes(self))


bass.Bass.to_json_bytes = _patched_to_json_bytes

# ----- problem constants (hardcoded per contract) -----
N = 50000          # nodes per side
F = 128            # feature dim
R = 5              # ratings
E = 400000         # edges per rating per direction
OUT = 64           # output dim
NB = 2             # basis count
NCORES = 8

WIN = 32768        # dma_gather index window (signed int16)
GCAP = 7           # max 128-chunks per dma_gather (num_idxs < 1024)

f32 = mybir.dt.float32
bf16 = mybir.dt.bfloat16
i32 = mybir.dt.int32
i16 = mybir.dt.int16


def _derived():
    npc = N // NCORES
    nt = (npc + 127) // 128
    npad = nt * 128
    tbl = ((N + 1 + 127) // 128) * 128  # >= N+1 so row N exists and is zero
    return npc, nt, npad, tbl


def _splits(nchunks):
    """Split nchunks into groups of <= GCAP chunks: [(offset, count), ...]."""
    out = []
    o = 0
    while o < nchunks:
        g = min(GCAP, nchunks - o)
        out.append((o, g))
        o += g
    return out


# ======================================================================
# Host-side edge preprocessing
# ======================================================================

def _prep_direction(src_all, dst_all, C0, C1):
    """For one direction, build per-core gather-index streams + dstloc.

    Edges of each (rating, dest-tile) are split between two table windows
    (bases 0 and W1=TBL-WIN) by source id, padded to static chunk counts
    C0/C1 per rating.  If C0 is None, only computes the needed maxima
    (returns (maxC0, maxC1, maxC)).

    Returns (idx0, idx1, dstloc) with
      idx0: int16 [NCORES, 128, NT*R*C0*8]   window-0 gather indices (wrap16)
      idx1: int16 [NCORES, 128, NT*R*C1*8]
      dstloc: bf16 [NCORES, 128, NT*R*(C0+C1)]  dest offset / 200 for pads
    m-tile column layout per tile t: [r-major (r, j<C0) | r-major (r, j<C1)].
    """
    NPC, NT, _, TBLR = _derived()
    W1 = TBLR - WIN
    measure = C0 is None
    maxn0 = maxn1 = maxtot = 1
    if not measure:
        C0T, C1T = R * C0, R * C1
        CT = C0T + C1T
        idx0 = np.zeros((NCORES, NT * C0T * 128), np.int32)
        idx1 = np.zeros((NCORES, NT * C1T * 128), np.int32)
        dstloc = np.full((NCORES, NT * CT, 128), 200.0, np.float32)
    for r in range(R):
        order = np.argsort(dst_all[r], kind="stable")
        dst_s = dst_all[r][order].astype(np.int64)
        src_s = src_all[r][order].astype(np.int64)
        bounds = np.searchsorted(dst_s, np.arange(NCORES + 1) * NPC)
        for c in range(NCORES):
            lo, hi = bounds[c], bounds[c + 1]
            d = dst_s[lo:hi] - c * NPC
            s = src_s[lo:hi]
            t_id = d >> 7
            # category: 0 = must-window0 (s<W1), 1 = either, 2 = must-window1
            # final sort key includes src so each chunk's gather descriptors
            # walk the table monotonically (DRAM row locality)
            cat = np.where(s < W1, 0, np.where(s < WIN, 1, 2))
            key = t_id * 3 + cat
            o2 = np.argsort(key << 17 | s, kind="stable")
            d, s, t_id, cat, key = d[o2], s[o2], t_id[o2], cat[o2], key[o2]
            cnt = np.bincount(key, minlength=NT * 3).reshape(NT, 3)
            n0m, nov, n1m = cnt[:, 0], cnt[:, 1], cnt[:, 2]
            tot = n0m + nov + n1m
            maxn0 = max(maxn0, int(n0m.max()))
            maxn1 = max(maxn1, int(n1m.max()))
            maxtot = max(maxtot, int(tot.max()))
            if measure:
                continue
            # assign overlap edges: fill window-0 only as much as needed to
            # keep window-1 within its budget, and never beyond C0*128
            need = np.clip(tot - C1 * 128 - n0m, 0, None)
            a = np.clip(need, 0, np.minimum(nov, C0 * 128 - n0m))
            h0_sz = n0m + a
            assert (h0_sz <= C0 * 128).all() and (tot - h0_sz <= C1 * 128).all()
            t_start = np.zeros(NT + 1, np.int64)
            np.cumsum(tot, out=t_start[1:])
            pos = np.arange(len(d)) - t_start[t_id]
            in_h0 = pos < h0_sz[t_id]
            # window 0
            rank = pos[in_h0]
            tt = t_id[in_h0]
            j = rank >> 7
            cc = r * C0 + j
            q = (tt * C0T + cc) * 128 + (rank & 127)
            idx0[c][q] = s[in_h0]
            dstloc[c][tt * CT + cc, rank & 127] = d[in_h0] & 127
            # window 1
            rank = (pos - h0_sz[t_id])[~in_h0]
            tt = t_id[~in_h0]
            j = rank >> 7
            cc = r * C1 + j
            q = (tt * C1T + cc) * 128 + (rank & 127)
            idx1[c][q] = s[~in_h0] - W1
            dstloc[c][tt * CT + C0T + cc, rank & 127] = d[~in_h0] & 127
    if measure:
        return maxn0, maxn1, maxtot

    def wrap16(a):
        # flat [cores, L] -> [cores, 128, L//16] (wrapped, replicated 8x)
        L = a.shape[1]
        w = a.reshape(NCORES, L // 16, 16).transpose(0, 2, 1)  # [c, 16, L/16]
        return np.ascontiguousarray(
            np.broadcast_to(w[:, None, :, :], (NCORES, 8, 16, L // 16))
            .reshape(NCORES, 128, L // 16)).astype(np.int16)

    dstloc = np.ascontiguousarray(
        dstloc.transpose(0, 2, 1)).astype(BF16)  # [c, 128, NT*CT]
    return wrap16(idx0), wrap16(idx1), dstloc


# ======================================================================
# Launch 1: build bf16 gather tables (x = cj * feat), row-sharded
# ======================================================================

def build_prep_nc():
    NPC, NT, NPAD, TBL = _derived()
    nc = bass.Bass()
    feat_in = nc.dram_tensor("feat_slice", (2, NPAD, F), f32, kind="ExternalInput")
    # cj host-transposed to [2, 128, NT]: element [s, i, t] = cj[s, t*128+i]
    cj_in = nc.dram_tensor("cj_slice", (2, 128, NT), f32, kind="ExternalInput")
    x_out = nc.dram_tensor("x_slice", (2, NPAD, F), bf16, kind="ExternalOutput")

    with tile.TileContext(nc) as tc:
        with (
            tc.tile_pool(name="cj", bufs=1) as cjp,
            tc.tile_pool(name="sb", bufs=6) as sb,
        ):
            cj_sb = cjp.tile([128, 2 * NT], f32, tag="cj")
            nc.sync.dma_start(
                out=cj_sb[:].rearrange("p (s t) -> p s t", s=2),
                in_=cj_in[:, :, :].rearrange("s p t -> p s t"))
            # absorber: advance DVE's clock past the cj DMA so later consumers
            # need only one wait (walrus allows a single sync wait per compute
            # instruction and the scheduler doesn't always split).
            scratch = cjp.tile([128, 1], f32, tag="scratch")
            nc.vector.tensor_copy(out=scratch[:], in_=cj_sb[:, :1])
            for side in range(2):
                for t in range(NT):
                    rows = slice(t * 128, (t + 1) * 128)
                    ft = sb.tile([128, F], f32, tag="ft")
                    nc.sync.dma_start(out=ft[:], in_=feat_in[side, rows, :])
                    xt = sb.tile([128, F], bf16, tag="xt")
                    c0 = side * NT + t
                    nc.vector.tensor_tensor(
                        out=xt[:], in0=ft[:],
                        in1=cj_sb[:, c0:c0 + 1].to_broadcast([128, F]),
                        op=mybir.AluOpType.mult,
                    )
                    nc.sync.dma_start(out=x_out[side, rows, :], in_=xt[:])
    return nc


# ======================================================================
# Launch 2: main kernel
# ======================================================================

def build_main_nc(C0: int, C1: int, bench: bool = False):
    """bench=True: identical device work, but the big result goes to an
    Internal DRAM tensor and only a tiny tensor is an ExternalOutput --
    keeps PJRT from shipping 16MB/core back per call, so wall-clock
    differencing isolates device exec time."""
    NPC, NT, NPAD, TBL = _derived()
    W1 = TBL - WIN
    C0T, C1T = R * C0, R * C1
    CT = C0T + C1T
    nc = bass.Bass()
    x_drug = nc.dram_tensor("x_drug", (TBL, F), bf16, kind="ExternalInput")
    x_dis = nc.dram_tensor("x_dis", (TBL, F), bf16, kind="ExternalInput")
    idx0_in = nc.dram_tensor("idx0", (2, 128, NT * C0T * 8), i16, kind="ExternalInput")
    idx1_in = nc.dram_tensor("idx1", (2, 128, NT * C1T * 8), i16, kind="ExternalInput")
    dstloc_in = nc.dram_tensor("dstloc", (2, 128, NT * CT), bf16, kind="ExternalInput")
    # ci host-transposed to [2, 128, NT]
    ci_in = nc.dram_tensor("ci_pad", (2, 128, NT), f32, kind="ExternalInput")
    att_in = nc.dram_tensor("att", (R, NB), f32, kind="ExternalInput")
    basis_in = nc.dram_tensor("basis", (NB, F, F), f32, kind="ExternalInput")
    fcw_in = nc.dram_tensor("fc_w", (F, OUT), f32, kind="ExternalInput")
    fcb_in = nc.dram_tensor("fc_b", (OUT,), f32, kind="ExternalInput")
    if bench:
        out = nc.dram_tensor("out_scratch", (2, NPAD, R, OUT), f32,
                             kind="Internal")
        bench_out = nc.dram_tensor("bench_out", (128, R * OUT), f32,
                                   kind="ExternalOutput")
    else:
        out = nc.dram_tensor("out_part", (2, NPAD, R, OUT), f32,
                             kind="ExternalOutput")

    iota_np = np.broadcast_to(np.arange(128, dtype=np.float32), (128, 128))
    iota_c = nc.inline_tensor(np.ascontiguousarray(iota_np), "iota_c")
    ident_c = nc.inline_tensor(np.eye(128, dtype=np.float32), "ident_c")
    ones_c = nc.inline_tensor(np.ones((1, 128), dtype=np.float32), "ones_c")

    eq = mybir.AluOpType.is_equal
    mult = mybir.AluOpType.mult
    add = mybir.AluOpType.add

    splits0 = _splits(C0T)
    splits1 = _splits(C1T)

    with tile.TileContext(nc) as tc:
        with (
            tc.tile_pool(name="const", bufs=1) as cp,
            tc.tile_pool(name="sb", bufs=4) as sb,
            tc.tile_pool(name="mp", bufs=3) as mp,
            tc.tile_pool(name="pp", bufs=3) as pp,
            tc.tile_pool(name="zp", bufs=4) as zp,
            tc.tile_pool(name="idxp", bufs=1) as idxp,
            tc.tile_pool(name="ps", bufs=4, space="PSUM") as ps,
            tc.tile_pool(name="ps2", bufs=4, space="PSUM") as ps2,
        ):
            nc.gpsimd.load_library(library_config.mlp)
            # one Pool register per distinct gather size (to_reg per call
            # would exhaust the register file across ~700 gathers)
            ni_regs = {gn: nc.gpsimd.to_reg(gn * 128)
                       for (_, gn) in set(splits0) | set(splits1)}
            # ---------- consts ----------
            iota_f = cp.tile([128, 128], f32, tag="iotaf")
            nc.sync.dma_start(out=iota_f[:], in_=iota_c[:, :])
            iota_t = cp.tile([128, 128], bf16, tag="iota")
            nc.vector.tensor_copy(out=iota_t[:], in_=iota_f[:])
            ident_t = cp.tile([128, 128], f32, tag="ident")
            nc.sync.dma_start(out=ident_t[:], in_=ident_c[:, :])
            ones_f32 = cp.tile([1, 128], f32, tag="ones32")
            nc.sync.dma_start(out=ones_f32[:], in_=ones_c[:, :])
            fcw_t = cp.tile([128, OUT], f32, tag="fcw")
            nc.sync.dma_start(out=fcw_t[:], in_=fcw_in[:, :])
            fcb_row = cp.tile([1, OUT], f32, tag="fcbrow")
            nc.sync.dma_start(out=fcb_row[:], in_=fcb_in[None, :])
            att_row = cp.tile([1, R * NB], f32, tag="attrow")
            nc.sync.dma_start(out=att_row[:],
                              in_=att_in[:, :].rearrange("r b -> () (r b)"))

            # ---------- W prep ----------
            attb_ps = ps2.tile([128, R * NB], f32, tag="o2")
            nc.tensor.matmul(out=attb_ps[:], lhsT=ones_f32[:], rhs=att_row[:],
                             start=True, stop=True)
            att_b = cp.tile([128, R * NB], f32, tag="attb")
            nc.vector.tensor_copy(out=att_b[:], in_=attb_ps[:])

            biasb_ps = ps2.tile([128, OUT], f32, tag="o2")
            nc.tensor.matmul(out=biasb_ps[:], lhsT=ones_f32[:], rhs=fcb_row[:],
                             start=True, stop=True)
            bias5 = cp.tile([128, R * OUT], f32, tag="bias5")
            for r in range(R):
                nc.vector.tensor_copy(out=bias5[:, r * OUT:(r + 1) * OUT],
                                      in_=biasb_ps[:])

            # basis[b] transposed: [e, f]
            bT = []
            for b in range(NB):
                bt_in = sb.tile([128, 128], f32, tag="bload")
                nc.sync.dma_start(out=bt_in[:], in_=basis_in[b, :, :])
                bt_ps = ps.tile([128, 128], f32, tag="zt")
                nc.tensor.transpose(out=bt_ps[:], in_=bt_in[:], identity=ident_t[:])
                bt_sb = cp.tile([128, 128], f32, tag=f"bT{b}")
                nc.vector.tensor_copy(out=bt_sb[:], in_=bt_ps[:])
                bT.append(bt_sb)

            wfc = cp.tile([128, R * OUT], bf16, tag="wfc")
            for r in range(R):
                wrt = sb.tile([128, 128], f32, tag="wrt")
                tmp = sb.tile([128, 128], f32, tag="wtmp")
                nc.vector.tensor_tensor(
                    out=tmp[:], in0=bT[1][:],
                    in1=att_b[:, 2 * r + 1:2 * r + 2].to_broadcast([128, 128]),
                    op=mult,
                )
                nc.vector.tensor_tensor(
                    out=wrt[:], in0=bT[0][:],
                    in1=att_b[:, 2 * r:2 * r + 1].to_broadcast([128, 128]),
                    op=mult,
                )
                nc.vector.tensor_tensor(out=wrt[:], in0=wrt[:], in1=tmp[:], op=add)
                wfc_ps = ps2.tile([128, OUT], f32, tag="o2")
                nc.tensor.matmul(out=wfc_ps[:], lhsT=wrt[:], rhs=fcw_t[:],
                                 start=True, stop=True)
                nc.scalar.copy(out=wfc[:, r * OUT:(r + 1) * OUT], in_=wfc_ps[:])

            # ---------- main loops ----------
            ci_sb = cp.tile([128, 2 * NT], f32, tag="ci")
            nc.sync.dma_start(
                out=ci_sb[:].rearrange("p (s t) -> p s t", s=2),
                in_=ci_in[:, :, :].rearrange("s p t -> p s t"))
            for d in range(2):
                x_src = x_dis if d else x_drug
                idx0_t = idxp.tile([128, NT * C0T * 8], i16, tag="idx0")
                nc.sync.dma_start(out=idx0_t[:], in_=idx0_in[d, :, :])
                idx1_t = idxp.tile([128, NT * C1T * 8], i16, tag="idx1")
                nc.sync.dma_start(out=idx1_t[:], in_=idx1_in[d, :, :])
                dl_t = idxp.tile([128, NT * CT], bf16, tag="dl")
                nc.sync.dma_start(out=dl_t[:], in_=dstloc_in[d, :, :])
                for t in range(NT):
                    rows = slice(t * 128, (t + 1) * 128)
                    ci_col = d * NT + t
                    m_t = mp.tile([128, CT * 128], bf16, tag="m")
                    for (go, gn) in splits0:
                        ni = gn * 128
                        nc.gpsimd.dma_gather(
                            m_t[:, go * 128:(go + gn) * 128].rearrange(
                                "p (c w) -> p c w", w=F),
                            x_src[0:WIN, :],
                            idx0_t[:, (t * C0T + go) * 8:(t * C0T + go + gn) * 8],
                            ni, ni_regs[gn], F,
                        )
                    for (go, gn) in splits1:
                        ni = gn * 128
                        nc.gpsimd.dma_gather(
                            m_t[:, (C0T + go) * 128:(C0T + go + gn) * 128]
                            .rearrange("p (c w) -> p c w", w=F),
                            x_src[W1:W1 + WIN, :],
                            idx1_t[:, (t * C1T + go) * 8:(t * C1T + go + gn) * 8],
                            ni, ni_regs[gn], F,
                        )
                    p_t = pp.tile([128, CT * 128], bf16, tag="p")
                    nc.vector.tensor_tensor(
                        out=p_t[:].rearrange("p (c f) -> p c f", c=CT),
                        in0=dl_t[:, t * CT:(t + 1) * CT][:, :, None].to_broadcast(
                            [128, CT, 128]),
                        in1=iota_t[:, None, :].to_broadcast([128, CT, 128]),
                        op=eq,
                    )
                    o2 = ps2.tile([128, R * OUT], f32, tag="o2")
                    for r in range(R):
                        cols = ([r * C0 + j for j in range(C0)] +
                                [C0T + r * C1 + j for j in range(C1)])
                        zt = ps.tile([128, 128], f32, tag="zt")
                        for k, cc in enumerate(cols):
                            nc.tensor.matmul(
                                out=zt[:],
                                lhsT=m_t[:, cc * 128:(cc + 1) * 128],
                                rhs=p_t[:, cc * 128:(cc + 1) * 128],
                                start=(k == 0), stop=(k == len(cols) - 1),
                            )
                        zt_sb = zp.tile([128, 128], bf16, tag="ztsb")
                        nc.scalar.copy(out=zt_sb[:], in_=zt[:])
                        nc.tensor.matmul(
                            out=o2[:, r * OUT:(r + 1) * OUT], lhsT=zt_sb[:],
                            rhs=wfc[:, r * OUT:(r + 1) * OUT],
                            start=True, stop=True,
                        )
                    ob = sb.tile([128, R * OUT], f32, tag="ob")
                    nc.vector.tensor_tensor(
                        out=ob[:], in0=o2[:],
                        in1=ci_sb[:, ci_col:ci_col + 1].to_broadcast(
                            [128, R * OUT]),
                        op=mult,
                    )
                    nc.vector.tensor_tensor(
                        out=ob[:], in0=ob[:], in1=bias5[:], op=add)
                    nc.sync.dma_start(
                        out=out[d, rows, :, :].rearrange("p r o -> p (r o)"),
                        in_=ob[:],
                    )
                    if bench and d == 1 and t == NT - 1:
                        nc.sync.dma_start(out=bench_out[:, :], in_=ob[:])
    mybir.codegen_inst_isa_subclasses(nc)
    return nc


# ======================================================================
# kernel entry
# ======================================================================

_cache: dict = {}


def kernel(drug_feat, dis_feat, cj_drug, ci_drug, cj_dis, ci_dis,
           att, basis, fc_w, fc_b, edge_drug, edge_dis):
    NPC, NT, NPAD, TBL = _derived()
    drug_feat = np.asarray(drug_feat, np.float32)
    dis_feat = np.asarray(dis_feat, np.float32)
    cj_drug = np.asarray(cj_drug, np.float32)
    ci_drug = np.asarray(ci_drug, np.float32)
    cj_dis = np.asarray(cj_dis, np.float32)
    ci_dis = np.asarray(ci_dis, np.float32)
    att = np.asarray(att, np.float32)
    basis = np.asarray(basis, np.float32)
    fc_w = np.asarray(fc_w, np.float32)
    fc_b = np.asarray(fc_b, np.float32)
    edge_drug = np.asarray(edge_drug, np.int32)
    edge_dis = np.asarray(edge_dis, np.int32)

    # ---- host preprocessing: edge sort/shard (index manipulation only) ----
    # direction 0: drug -> dis (dest = dis), direction 1: dis -> drug
    _tlog("start")
    n0a, n1a, ta = _prep_direction(edge_drug, edge_dis, None, None)
    n0b, n1b, tb = _prep_direction(edge_dis, edge_drug, None, None)
    C0 = (max(n0a, n0b) + 127) // 128
    Cm = (max(ta, tb) + 127) // 128
    C1 = max((max(n1a, n1b) + 127) // 128, Cm - C0)
    idx0_d0, idx1_d0, dl_d0 = _prep_direction(edge_drug, edge_dis, C0, C1)
    idx0_d1, idx1_d1, dl_d1 = _prep_direction(edge_dis, edge_drug, C0, C1)
    _tlog(f"host prep done C0={C0} C1={C1}")

    # ---- launch 1: build gather tables ----
    if "prep" not in _cache:
        _cache["prep"] = build_prep_nc()
    nc1 = _cache["prep"]

    in_maps1 = []
    for c in range(NCORES):
        rows = slice(c * NPC, (c + 1) * NPC)
        feat_slice = np.zeros((2, NPAD, F), np.float32)
        feat_slice[0, :NPC] = drug_feat[rows]
        feat_slice[1, :NPC] = dis_feat[rows]
        cj_slice = np.zeros((2, NPAD), np.float32)
        cj_slice[0, :NPC] = cj_drug[rows]
        cj_slice[1, :NPC] = cj_dis[rows]
        cj_slice = np.ascontiguousarray(
            cj_slice.reshape(2, NT, 128).transpose(0, 2, 1))
        in_maps1.append({"feat_slice": feat_slice, "cj_slice": cj_slice})
    _tlog("launch1 inputs built")
    res1 = run_bass_kernel_spmd(nc1, in_maps1, core_ids=list(range(NCORES)))
    _tlog("launch1 done")
    xs = [r["x_slice"] for r in res1.results]
    x_drug_full = np.zeros((TBL, F), BF16)
    x_dis_full = np.zeros((TBL, F), BF16)
    for c in range(NCORES):
        rows = slice(c * NPC, (c + 1) * NPC)
        x_drug_full[rows] = xs[c][0, :NPC]
        x_dis_full[rows] = xs[c][1, :NPC]

    # ---- launch 2: main ----
    global _last_cfg
    _last_cfg = (C0, C1)
    key = ("main", C0, C1)
    if key not in _cache:
        _cache[key] = build_main_nc(C0, C1)
    nc2 = _cache[key]

    in_maps2 = []
    for c in range(NCORES):
        rows = slice(c * NPC, (c + 1) * NPC)
        ci_pad = np.zeros((2, NPAD), np.float32)
        ci_pad[0, :NPC] = ci_dis[rows]    # dir 0 dest = dis
        ci_pad[1, :NPC] = ci_drug[rows]   # dir 1 dest = drug
        ci_pad = np.ascontiguousarray(
            ci_pad.reshape(2, NT, 128).transpose(0, 2, 1))
        in_maps2.append({
            "x_drug": x_drug_full, "x_dis": x_dis_full,
            "idx0": np.stack([idx0_d0[c], idx0_d1[c]], axis=0),
            "idx1": np.stack([idx1_d0[c], idx1_d1[c]], axis=0),
            "dstloc": np.stack([dl_d0[c], dl_d1[c]], axis=0),
            "ci_pad": ci_pad,
            "att": att, "basis": basis, "fc_w": fc_w, "fc_b": fc_b,
        })
    _tlog("launch2 inputs built")
    res2 = run_bass_kernel_spmd(nc2, in_maps2, core_ids=list(range(NCORES)))
    _tlog("launch2 done")

    out_dis = np.concatenate(
        [r["out_part"][0, :NPC] for r in res2.results], axis=0)
    out_drug = np.concatenate(
        [r["out_part"][1, :NPC] for r in res2.results], axis=0)
    _tlog("assembled")
    return out_drug.astype(np.float32), out_dis.astype(np.float32)


# revision 16
# speedup vs baseline: 6.0799x; 1.8738x over previous
"""GCMC layer (gnn_message_passing) Bass kernel for 8 Trainium2 NeuronCores.

Strategy (dest-sharded, no collectives):
  out_dis[m, r, :] = ci_dis[m] * (S_dis[r][m] @ Wfc_r) + fc_b
  where S_dis[r][m] = sum_{edges e of rating r with dst=m} x_drug[src[e]]
        x_drug[n]   = cj_drug[n] * drug_feat[n]      (bf16 gather table)
        Wfc_r       = (sum_b att[r,b]*basis[b]) @ fc_w    [F, OUT]
  (and symmetrically for the reverse direction dis->drug)

  - Host sorts edges of each (direction, rating) by destination, shards
    destinations across 8 cores, groups them into dest tiles of 128, and
    lays out per-tile edge chunks of 128 padded to a static chunk count.
  - Launch 1: each core scales its 1/8 slice of node features by cj -> bf16.
    Host concatenates the slices into full gather tables.
  - Launch 2 (main): per dest tile: batched SWDGE gathers (InstDMAGatherAnt,
    up to 7x128 rows per instruction -- the num_idxs field tops out below
    1024) fetch all the tile's message rows; DVE builds the one-hot
    P = is_equal(dstloc, iota); TensorE accumulates ZT[f,d] += M.T @ P in
    PSUM (the segment sum), a second matmul ZT.T @ Wfc_r lands in a
    per-tile [128, R*OUT] PSUM bank, DVE applies ci scale + bias, and the
    result is stored contiguously in the final [node, r, out] layout.
  - dma_gather indices are signed int16 (< 32768), so each gather reads
    through one of two overlapping 32768-row windows of the x table
    (bases 0 and TBL-32768).  Edges are assigned to a window by source id:
    src < 17280 must use window 0, src >= 32768 must use window 1, and the
    overlap is used as slack to balance the two static chunk budgets.
    Pad slots point at a harmless valid row; their dstloc is 200 so the
    one-hot match never fires and they contribute nothing.
"""

import json
import os
import time

import numpy as np
import ml_dtypes

_VERBOSE = os.environ.get("KERNEL_VERBOSE", "0") == "1"


def _tlog(msg, t0=[None]):
    if _VERBOSE:
        now = time.time()
        dt = 0.0 if t0[0] is None else now - t0[0]
        t0[0] = now
        print(f"[kernel +{dt:6.2f}s] {msg}", flush=True)

import concourse.bass as bass
import concourse.mybir as mybir
import concourse.tile as tile
from concourse import library_config
from concourse.bass_utils import run_bass_kernel_spmd

BF16 = ml_dtypes.bfloat16


# ----------------------------------------------------------------------
# Workaround: the staged walrus rejects >1 sync wait per instruction
# ("Too many sync wait commands") while the Tile scheduler emits multi-wait
# instructions.  Split extra waits into standalone EventSemaphore
# instructions right before the owning instruction (same engine queue, so
# semantics are identical: all waits are pre-conditions).
# ----------------------------------------------------------------------

def _split_multiwaits(bir: bytes) -> bytes:
    j = json.loads(bir)
    for fn in j["functions"]:
        for blk in fn["blocks"]:
            out = []
            k = 0
            for ins in blk["instructions"]:
                si = ins.get("sync_info") or {}
                waits = si.get("on_wait") or []
                if len(waits) > 1:
                    for w in waits[:-1]:
                        out.append({
                            "debug": ins.get("debug"),
                            "engine": ins["engine"],
                            "ins": [], "outs": [],
                            "name": f"{ins['name']}-ws{k}",
                            "opcode": "EventSemaphore",
                            "sync_info": {"on_update": [], "on_wait": [w]},
                        })
                        k += 1
                    si["on_wait"] = [waits[-1]]
                out.append(ins)
            blk["instructions"] = out
    return json.dumps(j).encode()


_orig_to_json_bytes = bass.Bass.to_json_bytes


def _patched_to_json_bytes(self):
    return _split_multiwaits(_orig_to_json_bytes(self))


bass.Bass.to_json_bytes = _patched_to_json_bytes

# ----- problem constants (hardcoded per contract) -----
N = 50000          # nodes per side
F = 128            # feature dim
R = 5              # ratings
E = 400000         # edges per rating per direction
OUT = 64           # output dim
NB = 2             # basis count
NCORES = 8

WIN = 32768        # dma_gather index window (signed int16)
GCAP = 7           # max 128-chunks per dma_gather (num_idxs < 1024)

f32 = mybir.dt.float32
bf16 = mybir.dt.bfloat16
i32 = mybir.dt.int32
i16 = mybir.dt.int16


def _derived():
    npc = N // NCORES
    nt = (npc + 127) // 128
    npad = nt * 128
    tbl = ((N + 1 + 127) // 128) * 128  # >= N+1 so row N exists and is zero
    return npc, nt, npad, tbl


def _splits(nchunks):
    """Split nchunks into groups of <= GCAP chunks: [(offset, count), ...]."""
    out = []
    o = 0
    while o < nchunks:
        g = min(GCAP, nchunks - o)
        out.append((o, g))
        o += g
    return out


# ======================================================================
# Host-side edge preprocessing
# ======================================================================

def _prep_direction(src_all, dst_all, C0, C1):
    """For one direction, build per-core gather-index streams + dstloc.

    Edges of each (rating, dest-tile) are split between two table windows
    (bases 0 and W1=TBL-WIN) by source id, padded to static chunk counts
    C0/C1 per rating.  If C0 is None, only computes the needed maxima
    (returns (maxC0, maxC1, maxC)).

    Returns (idx0, idx1, dstloc) with
      idx0: int16 [NCORES, 128, NT*R*C0*8]   window-0 gather indices (wrap16)
      idx1: int16 [NCORES, 128, NT*R*C1*8]
      dstloc: bf16 [NCORES, 128, NT*R*(C0+C1)]  dest offset / 200 for pads
    m-tile column layout per tile t: [r-major (r, j<C0) | r-major (r, j<C1)].
    """
    NPC, NT, _, TBLR = _derived()
    W1 = TBLR - WIN
    measure = C0 is None
    maxn0 = maxn1 = maxtot = 1
    if not measure:
        C0T, C1T = R * C0, R * C1
        CT = C0T + C1T
        idx0 = np.zeros((NCORES, NT * C0T * 128), np.int32)
        idx1 = np.zeros((NCORES, NT * C1T * 128), np.int32)
        dstloc = np.full((NCORES, NT * CT, 128), 200.0, np.float32)
    for r in range(R):
        order = np.argsort(dst_all[r], kind="stable")
        dst_s = dst_all[r][order].astype(np.int64)
        src_s = src_all[r][order].astype(np.int64)
        bounds = np.searchsorted(dst_s, np.arange(NCORES + 1) * NPC)
        for c in range(NCORES):
            lo, hi = bounds[c], bounds[c + 1]
            d = dst_s[lo:hi] - c * NPC
            s = src_s[lo:hi]
            t_id = d >> 7
            # category: 0 = must-window0 (s<W1), 1 = either, 2 = must-window1
            # final sort key includes src so each chunk's gather descriptors
            # walk the table monotonically (DRAM row locality)
            cat = np.where(s < W1, 0, np.where(s < WIN, 1, 2))
            key = t_id * 3 + cat
            o2 = np.argsort(key << 17 | s, kind="stable")
            d, s, t_id, cat, key = d[o2], s[o2], t_id[o2], cat[o2], key[o2]
            cnt = np.bincount(key, minlength=NT * 3).reshape(NT, 3)
            n0m, nov, n1m = cnt[:, 0], cnt[:, 1], cnt[:, 2]
            tot = n0m + nov + n1m
            maxn0 = max(maxn0, int(n0m.max()))
            maxn1 = max(maxn1, int(n1m.max()))
            maxtot = max(maxtot, int(tot.max()))
            if measure:
                continue
            # assign overlap edges: fill window-0 only as much as needed to
            # keep window-1 within its budget, and never beyond C0*128
            need = np.clip(tot - C1 * 128 - n0m, 0, None)
            a = np.clip(need, 0, np.minimum(nov, C0 * 128 - n0m))
            h0_sz = n0m + a
            assert (h0_sz <= C0 * 128).all() and (tot - h0_sz <= C1 * 128).all()
            t_start = np.zeros(NT + 1, np.int64)
            np.cumsum(tot, out=t_start[1:])
            pos = np.arange(len(d)) - t_start[t_id]
            in_h0 = pos < h0_sz[t_id]
            # window 0
            rank = pos[in_h0]
            tt = t_id[in_h0]
            j = rank >> 7
            cc = r * C0 + j
            q = (tt * C0T + cc) * 128 + (rank & 127)
            idx0[c][q] = s[in_h0]
            dstloc[c][tt * CT + cc, rank & 127] = d[in_h0] & 127
            # window 1
            rank = (pos - h0_sz[t_id])[~in_h0]
            tt = t_id[~in_h0]
            j = rank >> 7
            cc = r * C1 + j
            q = (tt * C1T + cc) * 128 + (rank & 127)
            idx1[c][q] = s[~in_h0] - W1
            dstloc[c][tt * CT + C0T + cc, rank & 127] = d[~in_h0] & 127
    if measure:
        return maxn0, maxn1, maxtot

    def wrap16(a):
        # flat [cores, L] -> [cores, 128, L//16] (wrapped, replicated 8x)
        L = a.shape[1]
        w = a.reshape(NCORES, L // 16, 16).transpose(0, 2, 1)  # [c, 16, L/16]
        return np.ascontiguousarray(
            np.broadcast_to(w[:, None, :, :], (NCORES, 8, 16, L // 16))
            .reshape(NCORES, 128, L // 16)).astype(np.int16)

    dstloc = np.ascontiguousarray(
        dstloc.transpose(0, 2, 1)).astype(BF16)  # [c, 128, NT*CT]
    return wrap16(idx0), wrap16(idx1), dstloc


# ======================================================================
# Launch 1: build bf16 gather tables (x = cj * feat), row-sharded
# ======================================================================

def build_prep_nc():
    NPC, NT, NPAD, TBL = _derived()
    nc = bass.Bass()
    feat_in = nc.dram_tensor("feat_slice", (2, NPAD, F), f32, kind="ExternalInput")
    # cj host-transposed to [2, 128, NT]: element [s, i, t] = cj[s, t*128+i]
    cj_in = nc.dram_tensor("cj_slice", (2, 128, NT), f32, kind="ExternalInput")
    x_out = nc.dram_tensor("x_slice", (2, NPAD, F), bf16, kind="ExternalOutput")

    with tile.TileContext(nc) as tc:
        with (
            tc.tile_pool(name="cj", bufs=1) as cjp,
            tc.tile_pool(name="sb", bufs=6) as sb,
        ):
            cj_sb = cjp.tile([128, 2 * NT], f32, tag="cj")
            nc.sync.dma_start(
                out=cj_sb[:].rearrange("p (s t) -> p s t", s=2),
                in_=cj_in[:, :, :].rearrange("s p t -> p s t"))
            # absorber: advance DVE's clock past the cj DMA so later consumers
            # need only one wait (walrus allows a single sync wait per compute
            # instruction and the scheduler doesn't always split).
            scratch = cjp.tile([128, 1], f32, tag="scratch")
            nc.vector.tensor_copy(out=scratch[:], in_=cj_sb[:, :1])
            for side in range(2):
                for t in range(NT):
                    rows = slice(t * 128, (t + 1) * 128)
                    ft = sb.tile([128, F], f32, tag="ft")
                    nc.sync.dma_start(out=ft[:], in_=feat_in[side, rows, :])
                    xt = sb.tile([128, F], bf16, tag="xt")
                    c0 = side * NT + t
                    nc.vector.tensor_tensor(
                        out=xt[:], in0=ft[:],
                        in1=cj_sb[:, c0:c0 + 1].to_broadcast([128, F]),
                        op=mybir.AluOpType.mult,
                    )
                    nc.sync.dma_start(out=x_out[side, rows, :], in_=xt[:])
    return nc


# ======================================================================
# Launch 2: main kernel
# ======================================================================

def build_main_nc(C0: int, C1: int, bench: bool = False):
    """bench=True: identical device work, but the big result goes to an
    Internal DRAM tensor and only a tiny tensor is an ExternalOutput --
    keeps PJRT from shipping 16MB/core back per call, so wall-clock
    differencing isolates device exec time."""
    NPC, NT, NPAD, TBL = _derived()
    W1 = TBL - WIN
    C0T, C1T = R * C0, R * C1
    CT = C0T + C1T
    # 4 SWDGE queues (ucode max): round-robined gathers pipeline desc-gen
    # and transfers; measured ~4.1ns/descriptor vs ~7.2ns on one queue.
    nc = bass.Bass(num_swdge_queues=4)
    x_drug = nc.dram_tensor("x_drug", (TBL, F), bf16, kind="ExternalInput")
    x_dis = nc.dram_tensor("x_dis", (TBL, F), bf16, kind="ExternalInput")
    idx0_in = nc.dram_tensor("idx0", (2, 128, NT * C0T * 8), i16, kind="ExternalInput")
    idx1_in = nc.dram_tensor("idx1", (2, 128, NT * C1T * 8), i16, kind="ExternalInput")
    dstloc_in = nc.dram_tensor("dstloc", (2, 128, NT * CT), bf16, kind="ExternalInput")
    # ci host-transposed to [2, 128, NT]
    ci_in = nc.dram_tensor("ci_pad", (2, 128, NT), f32, kind="ExternalInput")
    att_in = nc.dram_tensor("att", (R, NB), f32, kind="ExternalInput")
    basis_in = nc.dram_tensor("basis", (NB, F, F), f32, kind="ExternalInput")
    fcw_in = nc.dram_tensor("fc_w", (F, OUT), f32, kind="ExternalInput")
    fcb_in = nc.dram_tensor("fc_b", (OUT,), f32, kind="ExternalInput")
    if bench:
        out = nc.dram_tensor("out_scratch", (2, NPAD, R, OUT), f32,
                             kind="Internal")
        bench_out = nc.dram_tensor("bench_out", (128, R * OUT), f32,
                                   kind="ExternalOutput")
    else:
        out = nc.dram_tensor("out_part", (2, NPAD, R, OUT), f32,
                             kind="ExternalOutput")

    iota_np = np.broadcast_to(np.arange(128, dtype=np.float32), (128, 128))
    iota_c = nc.inline_tensor(np.ascontiguousarray(iota_np), "iota_c")
    ident_c = nc.inline_tensor(np.eye(128, dtype=np.float32), "ident_c")
    ones_c = nc.inline_tensor(np.ones((1, 128), dtype=np.float32), "ones_c")

    eq = mybir.AluOpType.is_equal
    mult = mybir.AluOpType.mult
    add = mybir.AluOpType.add

    splits0 = _splits(C0T)
    splits1 = _splits(C1T)
    qctr = [0]

    with tile.TileContext(nc) as tc:
        with (
            tc.tile_pool(name="const", bufs=1) as cp,
            tc.tile_pool(name="sb", bufs=4) as sb,
            tc.tile_pool(name="mp", bufs=3) as mp,
            tc.tile_pool(name="pp", bufs=3) as pp,
            tc.tile_pool(name="zp", bufs=4) as zp,
            tc.tile_pool(name="idxp", bufs=1) as idxp,
            tc.tile_pool(name="ps", bufs=4, space="PSUM") as ps,
            tc.tile_pool(name="ps2", bufs=4, space="PSUM") as ps2,
        ):
            nc.gpsimd.load_library(library_config.mlp)
            # one Pool register per distinct gather size (to_reg per call
            # would exhaust the register file across ~700 gathers)
            ni_regs = {gn: nc.gpsimd.to_reg(gn * 128)
                       for (_, gn) in set(splits0) | set(splits1)}
            # ---------- consts ----------
            iota_f = cp.tile([128, 128], f32, tag="iotaf")
            nc.sync.dma_start(out=iota_f[:], in_=iota_c[:, :])
            iota_t = cp.tile([128, 128], bf16, tag="iota")
            nc.vector.tensor_copy(out=iota_t[:], in_=iota_f[:])
            ident_t = cp.tile([128, 128], f32, tag="ident")
            nc.sync.dma_start(out=ident_t[:], in_=ident_c[:, :])
            ones_f32 = cp.tile([1, 128], f32, tag="ones32")
            nc.sync.dma_start(out=ones_f32[:], in_=ones_c[:, :])
            fcw_t = cp.tile([128, OUT], f32, tag="fcw")
            nc.sync.dma_start(out=fcw_t[:], in_=fcw_in[:, :])
            fcb_row = cp.tile([1, OUT], f32, tag="fcbrow")
            nc.sync.dma_start(out=fcb_row[:], in_=fcb_in[None, :])
            att_row = cp.tile([1, R * NB], f32, tag="attrow")
            nc.sync.dma_start(out=att_row[:],
                              in_=att_in[:, :].rearrange("r b -> () (r b)"))

            # ---------- W prep ----------
            attb_ps = ps2.tile([128, R * NB], f32, tag="o2")
            nc.tensor.matmul(out=attb_ps[:], lhsT=ones_f32[:], rhs=att_row[:],
                             start=True, stop=True)
            att_b = cp.tile([128, R * NB], f32, tag="attb")
            nc.vector.tensor_copy(out=att_b[:], in_=attb_ps[:])

            biasb_ps = ps2.tile([128, OUT], f32, tag="o2")
            nc.tensor.matmul(out=biasb_ps[:], lhsT=ones_f32[:], rhs=fcb_row[:],
                             start=True, stop=True)
            bias5 = cp.tile([128, R * OUT], f32, tag="bias5")
            for r in range(R):
                nc.vector.tensor_copy(out=bias5[:, r * OUT:(r + 1) * OUT],
                                      in_=biasb_ps[:])

            # basis[b] transposed: [e, f]
            bT = []
            for b in range(NB):
                bt_in = sb.tile([128, 128], f32, tag="bload")
                nc.sync.dma_start(out=bt_in[:], in_=basis_in[b, :, :])
                bt_ps = ps.tile([128, 128], f32, tag="zt")
                nc.tensor.transpose(out=bt_ps[:], in_=bt_in[:], identity=ident_t[:])
                bt_sb = cp.tile([128, 128], f32, tag=f"bT{b}")
                nc.vector.tensor_copy(out=bt_sb[:], in_=bt_ps[:])
                bT.append(bt_sb)

            wfc = cp.tile([128, R * OUT], bf16, tag="wfc")
            for r in range(R):
                wrt = sb.tile([128, 128], f32, tag="wrt")
                tmp = sb.tile([128, 128], f32, tag="wtmp")
                nc.vector.tensor_tensor(
                    out=tmp[:], in0=bT[1][:],
                    in1=att_b[:, 2 * r + 1:2 * r + 2].to_broadcast([128, 128]),
                    op=mult,
                )
                nc.vector.tensor_tensor(
                    out=wrt[:], in0=bT[0][:],
                    in1=att_b[:, 2 * r:2 * r + 1].to_broadcast([128, 128]),
                    op=mult,
                )
                nc.vector.tensor_tensor(out=wrt[:], in0=wrt[:], in1=tmp[:], op=add)
                wfc_ps = ps2.tile([128, OUT], f32, tag="o2")
                nc.tensor.matmul(out=wfc_ps[:], lhsT=wrt[:], rhs=fcw_t[:],
                                 start=True, stop=True)
                nc.scalar.copy(out=wfc[:, r * OUT:(r + 1) * OUT], in_=wfc_ps[:])

            # ---------- main loops ----------
            ci_sb = cp.tile([128, 2 * NT], f32, tag="ci")
            nc.sync.dma_start(
                out=ci_sb[:].rearrange("p (s t) -> p s t", s=2),
                in_=ci_in[:, :, :].rearrange("s p t -> p s t"))
            for d in range(2):
                x_src = x_dis if d else x_drug
                idx0_t = idxp.tile([128, NT * C0T * 8], i16, tag="idx0")
                nc.sync.dma_start(out=idx0_t[:], in_=idx0_in[d, :, :])
                idx1_t = idxp.tile([128, NT * C1T * 8], i16, tag="idx1")
                nc.sync.dma_start(out=idx1_t[:], in_=idx1_in[d, :, :])
                dl_t = idxp.tile([128, NT * CT], bf16, tag="dl")
                nc.sync.dma_start(out=dl_t[:], in_=dstloc_in[d, :, :])
                for t in range(NT):
                    rows = slice(t * 128, (t + 1) * 128)
                    ci_col = d * NT + t
                    m_t = mp.tile([128, CT * 128], bf16, tag="m")
                    for (go, gn) in splits0:
                        ni = gn * 128
                        nc.gpsimd.dma_gather(
                            m_t[:, go * 128:(go + gn) * 128].rearrange(
                                "p (c w) -> p c w", w=F),
                            x_src[0:WIN, :],
                            idx0_t[:, (t * C0T + go) * 8:(t * C0T + go + gn) * 8],
                            ni, ni_regs[gn], F, queue_num=qctr[0] % 4,
                        )
                        qctr[0] += 1
                    for (go, gn) in splits1:
                        ni = gn * 128
                        nc.gpsimd.dma_gather(
                            m_t[:, (C0T + go) * 128:(C0T + go + gn) * 128]
                            .rearrange("p (c w) -> p c w", w=F),
                            x_src[W1:W1 + WIN, :],
                            idx1_t[:, (t * C1T + go) * 8:(t * C1T + go + gn) * 8],
                            ni, ni_regs[gn], F, queue_num=qctr[0] % 4,
                        )
                        qctr[0] += 1
                    p_t = pp.tile([128, CT * 128], bf16, tag="p")
                    nc.vector.tensor_tensor(
                        out=p_t[:].rearrange("p (c f) -> p c f", c=CT),
                        in0=dl_t[:, t * CT:(t + 1) * CT][:, :, None].to_broadcast(
                            [128, CT, 128]),
                        in1=iota_t[:, None, :].to_broadcast([128, CT, 128]),
                        op=eq,
                    )
                    o2 = ps2.tile([128, R * OUT], f32, tag="o2")
                    for r in range(R):
                        cols = ([r * C0 + j for j in range(C0)] +
                                [C0T + r * C1 + j for j in range(C1)])
                        zt = ps.tile([128, 128], f32, tag="zt")
                        for k, cc in enumerate(cols):
                            nc.tensor.matmul(
                                out=zt[:],
                                lhsT=m_t[:, cc * 128:(cc + 1) * 128],
                                rhs=p_t[:, cc * 128:(cc + 1) * 128],
                                start=(k == 0), stop=(k == len(cols) - 1),
                            )
                        zt_sb = zp.tile([128, 128], bf16, tag="ztsb")
                        nc.scalar.copy(out=zt_sb[:], in_=zt[:])
                        nc.tensor.matmul(
                            out=o2[:, r * OUT:(r + 1) * OUT], lhsT=zt_sb[:],
                            rhs=wfc[:, r * OUT:(r + 1) * OUT],
                            start=True, stop=True,
                        )
                    ob = sb.tile([128, R * OUT], f32, tag="ob")
                    nc.vector.tensor_tensor(
                        out=ob[:], in0=o2[:],
                        in1=ci_sb[:, ci_col:ci_col + 1].to_broadcast(
                            [128, R * OUT]),
                        op=mult,
                    )
                    nc.vector.tensor_tensor(
                        out=ob[:], in0=ob[:], in1=bias5[:], op=add)
                    nc.sync.dma_start(
                        out=out[d, rows, :, :].rearrange("p r o -> p (r o)"),
                        in_=ob[:],
                    )
                    if bench and d == 1 and t == NT - 1:
                        nc.sync.dma_start(out=bench_out[:, :], in_=ob[:])
    mybir.codegen_inst_isa_subclasses(nc)
    return nc


# ======================================================================
# kernel entry
# ======================================================================

_cache: dict = {}


def kernel(drug_feat, dis_feat, cj_drug, ci_drug, cj_dis, ci_dis,
           att, basis, fc_w, fc_b, edge_drug, edge_dis):
    NPC, NT, NPAD, TBL = _derived()
    drug_feat = np.asarray(drug_feat, np.float32)
    dis_feat = np.asarray(dis_feat, np.float32)
    cj_drug = np.asarray(cj_drug, np.float32)
    ci_drug = np.asarray(ci_drug, np.float32)
    cj_dis = np.asarray(cj_dis, np.float32)
    ci_dis = np.asarray(ci_dis, np.float32)
    att = np.asarray(att, np.float32)
    basis = np.asarray(basis, np.float32)
    fc_w = np.asarray(fc_w, np.float32)
    fc_b = np.asarray(fc_b, np.float32)
    edge_drug = np.asarray(edge_drug, np.int32)
    edge_dis = np.asarray(edge_dis, np.int32)

    # ---- host preprocessing: edge sort/shard (index manipulation only) ----
    # direction 0: drug -> dis (dest = dis), direction 1: dis -> drug
    _tlog("start")
    n0a, n1a, ta = _prep_direction(edge_drug, edge_dis, None, None)
    n0b, n1b, tb = _prep_direction(edge_dis, edge_drug, None, None)
    C0 = (max(n0a, n0b) + 127) // 128
    Cm = (max(ta, tb) + 127) // 128
    C1 = max((max(n1a, n1b) + 127) // 128, Cm - C0)
    idx0_d0, idx1_d0, dl_d0 = _prep_direction(edge_drug, edge_dis, C0, C1)
    idx0_d1, idx1_d1, dl_d1 = _prep_direction(edge_dis, edge_drug, C0, C1)
    _tlog(f"host prep done C0={C0} C1={C1}")

    # ---- launch 1: build gather tables ----
    if "prep" not in _cache:
        _cache["prep"] = build_prep_nc()
    nc1 = _cache["prep"]

    in_maps1 = []
    for c in range(NCORES):
        rows = slice(c * NPC, (c + 1) * NPC)
        feat_slice = np.zeros((2, NPAD, F), np.float32)
        feat_slice[0, :NPC] = drug_feat[rows]
        feat_slice[1, :NPC] = dis_feat[rows]
        cj_slice = np.zeros((2, NPAD), np.float32)
        cj_slice[0, :NPC] = cj_drug[rows]
        cj_slice[1, :NPC] = cj_dis[rows]
        cj_slice = np.ascontiguousarray(
            cj_slice.reshape(2, NT, 128).transpose(0, 2, 1))
        in_maps1.append({"feat_slice": feat_slice, "cj_slice": cj_slice})
    _tlog("launch1 inputs built")
    res1 = run_bass_kernel_spmd(nc1, in_maps1, core_ids=list(range(NCORES)))
    _tlog("launch1 done")
    xs = [r["x_slice"] for r in res1.results]
    x_drug_full = np.zeros((TBL, F), BF16)
    x_dis_full = np.zeros((TBL, F), BF16)
    for c in range(NCORES):
        rows = slice(c * NPC, (c + 1) * NPC)
        x_drug_full[rows] = xs[c][0, :NPC]
        x_dis_full[rows] = xs[c][1, :NPC]

    # ---- launch 2: main ----
    global _last_cfg
    _last_cfg = (C0, C1)
    key = ("main", C0, C1)
    if key not in _cache:
        _cache[key] = build_main_nc(C0, C1)
    nc2 = _cache[key]

    in_maps2 = []
    for c in range(NCORES):
        rows = slice(c * NPC, (c + 1) * NPC)
        ci_pad = np.zeros((2, NPAD), np.float32)
        ci_pad[0, :NPC] = ci_dis[rows]    # dir 0 dest = dis
        ci_pad[1, :NPC] = ci_drug[rows]   # dir 1 dest = drug
        ci_pad = np.ascontiguousarray(
            ci_pad.reshape(2, NT, 128).transpose(0, 2, 1))
        in_maps2.append({
            "x_drug": x_drug_full, "x_dis": x_dis_full,
            "idx0": np.stack([idx0_d0[c], idx0_d1[c]], axis=0),
            "idx1": np.stack([idx1_d0[c], idx1_d1[c]], axis=0),
            "dstloc": np.stack([dl_d0[c], dl_d1[c]], axis=0),
            "ci_pad": ci_pad,
            "att": att, "basis": basis, "fc_w": fc_w, "fc_b": fc_b,
        })
    _tlog("launch2 inputs built")
    res2 = run_bass_kernel_spmd(nc2, in_maps2, core_ids=list(range(NCORES)))
    _tlog("launch2 done")

    out_dis = np.concatenate(
        [r["out_part"][0, :NPC] for r in res2.results], axis=0)
    out_drug = np.concatenate(
        [r["out_part"][1, :NPC] for r in res2.results], axis=0)
    _tlog("assembled")
    return out_drug.astype(np.float32), out_dis.astype(np.float32)


# revision 18
# speedup vs baseline: 7.1317x; 1.1730x over previous
"""GCMC layer (gnn_message_passing) Bass kernel for 8 Trainium2 NeuronCores.

Strategy (dest-sharded, no collectives):
  out_dis[m, r, :] = ci_dis[m] * (S_dis[r][m] @ Wfc_r) + fc_b
  where S_dis[r][m] = sum_{edges e of rating r with dst=m} x_drug[src[e]]
        x_drug[n]   = cj_drug[n] * drug_feat[n]      (bf16 gather table)
        Wfc_r       = (sum_b att[r,b]*basis[b]) @ fc_w    [F, OUT]
  (and symmetrically for the reverse direction dis->drug)

  - Host sorts edges of each (direction, rating) by destination, shards
    destinations across 8 cores, groups them into dest tiles of 128, and
    lays out per-tile edge chunks of 128 padded to a static chunk count.
  - Launch 1: each core scales its 1/8 slice of node features by cj -> bf16.
    Host concatenates the slices into full gather tables.
  - Launch 2 (main): per dest tile: batched SWDGE gathers (InstDMAGatherAnt,
    up to 7x128 rows per instruction -- the num_idxs field tops out below
    1024) fetch all the tile's message rows; DVE builds the one-hot
    P = is_equal(dstloc, iota); TensorE accumulates ZT[f,d] += M.T @ P in
    PSUM (the segment sum), a second matmul ZT.T @ Wfc_r lands in a
    per-tile [128, R*OUT] PSUM bank, DVE applies ci scale + bias, and the
    result is stored contiguously in the final [node, r, out] layout.
  - dma_gather indices are signed int16 (< 32768), so each gather reads
    through one of two overlapping 32768-row windows of the x table
    (bases 0 and TBL-32768).  Edges are assigned to a window by source id:
    src < 17280 must use window 0, src >= 32768 must use window 1, and the
    overlap is used as slack to balance the two static chunk budgets.
    Pad slots point at a harmless valid row; their dstloc is 200 so the
    one-hot match never fires and they contribute nothing.
"""

import json
import os
import time

import numpy as np
import ml_dtypes

_VERBOSE = os.environ.get("KERNEL_VERBOSE", "0") == "1"


def _tlog(msg, t0=[None]):
    if _VERBOSE:
        now = time.time()
        dt = 0.0 if t0[0] is None else now - t0[0]
        t0[0] = now
        print(f"[kernel +{dt:6.2f}s] {msg}", flush=True)

import concourse.bass as bass
import concourse.mybir as mybir
import concourse.tile as tile
from concourse import library_config
from concourse.bass_utils import run_bass_kernel_spmd

BF16 = ml_dtypes.bfloat16


# ----------------------------------------------------------------------
# Workaround: the staged walrus rejects >1 sync wait per instruction
# ("Too many sync wait commands") while the Tile scheduler emits multi-wait
# instructions.  Split extra waits into standalone EventSemaphore
# instructions right before the owning instruction (same engine queue, so
# semantics are identical: all waits are pre-conditions).
# ----------------------------------------------------------------------

def _split_multiwaits(bir: bytes) -> bytes:
    j = json.loads(bir)
    for fn in j["functions"]:
        for blk in fn["blocks"]:
            out = []
            k = 0
            for ins in blk["instructions"]:
                si = ins.get("sync_info") or {}
                waits = si.get("on_wait") or []
                if len(waits) > 1:
                    for w in waits[:-1]:
                        out.append({
                            "debug": ins.get("debug"),
                            "engine": ins["engine"],
                            "ins": [], "outs": [],
                            "name": f"{ins['name']}-ws{k}",
                            "opcode": "EventSemaphore",
                            "sync_info": {"on_update": [], "on_wait": [w]},
                        })
                        k += 1
                    si["on_wait"] = [waits[-1]]
                out.append(ins)
            blk["instructions"] = out
    return json.dumps(j).encode()


_orig_to_json_bytes = bass.Bass.to_json_bytes


def _patched_to_json_bytes(self):
    return _split_multiwaits(_orig_to_json_bytes(self))


bass.Bass.to_json_bytes = _patched_to_json_bytes

# ----- problem constants (hardcoded per contract) -----
N = 50000          # nodes per side
F = 128            # feature dim
R = 5              # ratings
E = 400000         # edges per rating per direction
OUT = 64           # output dim
NB = 2             # basis count
NCORES = 8

WIN = 32768        # dma_gather index window (signed int16)
GCAP = 7           # max 128-chunks per dma_gather (num_idxs < 1024)

f32 = mybir.dt.float32
bf16 = mybir.dt.bfloat16
i32 = mybir.dt.int32
i16 = mybir.dt.int16


def _derived():
    npc = N // NCORES
    nt = (npc + 127) // 128
    npad = nt * 128
    tbl = ((N + 1 + 127) // 128) * 128  # >= N+1 so row N exists and is zero
    return npc, nt, npad, tbl


def _splits(nchunks):
    """Split nchunks into groups of <= GCAP chunks: [(offset, count), ...]."""
    out = []
    o = 0
    while o < nchunks:
        g = min(GCAP, nchunks - o)
        out.append((o, g))
        o += g
    return out


# ======================================================================
# Host-side edge preprocessing
# ======================================================================

def _prep_direction(src_all, dst_all, C0, C1):
    """For one direction, build per-core gather-index streams + dstloc.

    Edges of each (rating, dest-tile) are split between two table windows
    (bases 0 and W1=TBL-WIN) by source id, padded to static chunk counts
    C0/C1 per rating.  If C0 is None, only computes the needed maxima
    (returns (maxC0, maxC1, maxC)).

    Returns (idx0, idx1, dstloc) with
      idx0: int16 [NCORES, 128, NT*R*C0*8]   window-0 gather indices (wrap16)
      idx1: int16 [NCORES, 128, NT*R*C1*8]
      dstloc: bf16 [NCORES, 128, NT*R*(C0+C1)]  dest offset / 200 for pads
    m-tile column layout per tile t: [r-major (r, j<C0) | r-major (r, j<C1)].
    """
    NPC, NT, _, TBLR = _derived()
    W1 = TBLR - WIN
    measure = C0 is None
    maxn0 = maxn1 = maxtot = 1
    if not measure:
        C0T, C1T = R * C0, R * C1
        CT = C0T + C1T
        idx0 = np.zeros((NCORES, NT * C0T * 128), np.int32)
        idx1 = np.zeros((NCORES, NT * C1T * 128), np.int32)
        dstloc = np.full((NCORES, NT * CT, 128), 200.0, np.float32)
    for r in range(R):
        order = np.argsort(dst_all[r], kind="stable")
        dst_s = dst_all[r][order].astype(np.int64)
        src_s = src_all[r][order].astype(np.int64)
        bounds = np.searchsorted(dst_s, np.arange(NCORES + 1) * NPC)
        for c in range(NCORES):
            lo, hi = bounds[c], bounds[c + 1]
            d = dst_s[lo:hi] - c * NPC
            s = src_s[lo:hi]
            t_id = d >> 7
            # category: 0 = must-window0 (s<W1), 1 = either, 2 = must-window1
            # final sort key includes src so each chunk's gather descriptors
            # walk the table monotonically (DRAM row locality)
            cat = np.where(s < W1, 0, np.where(s < WIN, 1, 2))
            key = t_id * 3 + cat
            o2 = np.argsort(key << 17 | s, kind="stable")
            d, s, t_id, cat, key = d[o2], s[o2], t_id[o2], cat[o2], key[o2]
            cnt = np.bincount(key, minlength=NT * 3).reshape(NT, 3)
            n0m, nov, n1m = cnt[:, 0], cnt[:, 1], cnt[:, 2]
            tot = n0m + nov + n1m
            maxn0 = max(maxn0, int(n0m.max()))
            maxn1 = max(maxn1, int(n1m.max()))
            maxtot = max(maxtot, int(tot.max()))
            if measure:
                continue
            # assign overlap edges: fill window-0 only as much as needed to
            # keep window-1 within its budget, and never beyond C0*128
            need = np.clip(tot - C1 * 128 - n0m, 0, None)
            a = np.clip(need, 0, np.minimum(nov, C0 * 128 - n0m))
            h0_sz = n0m + a
            assert (h0_sz <= C0 * 128).all() and (tot - h0_sz <= C1 * 128).all()
            t_start = np.zeros(NT + 1, np.int64)
            np.cumsum(tot, out=t_start[1:])
            pos = np.arange(len(d)) - t_start[t_id]
            in_h0 = pos < h0_sz[t_id]
            # window 0
            rank = pos[in_h0]
            tt = t_id[in_h0]
            j = rank >> 7
            cc = r * C0 + j
            q = (tt * C0T + cc) * 128 + (rank & 127)
            idx0[c][q] = s[in_h0]
            dstloc[c][tt * CT + cc, rank & 127] = d[in_h0] & 127
            # window 1
            rank = (pos - h0_sz[t_id])[~in_h0]
            tt = t_id[~in_h0]
            j = rank >> 7
            cc = r * C1 + j
            q = (tt * C1T + cc) * 128 + (rank & 127)
            idx1[c][q] = s[~in_h0] - W1
            dstloc[c][tt * CT + C0T + cc, rank & 127] = d[~in_h0] & 127
    if measure:
        return maxn0, maxn1, maxtot

    def wrap16(a):
        # flat [cores, L] -> [cores, 128, L//16] (wrapped, replicated 8x)
        L = a.shape[1]
        w = a.reshape(NCORES, L // 16, 16).transpose(0, 2, 1)  # [c, 16, L/16]
        return np.ascontiguousarray(
            np.broadcast_to(w[:, None, :, :], (NCORES, 8, 16, L // 16))
            .reshape(NCORES, 128, L // 16)).astype(np.int16)

    dstloc = np.ascontiguousarray(
        dstloc.transpose(0, 2, 1)).astype(BF16)  # [c, 128, NT*CT]
    return wrap16(idx0), wrap16(idx1), dstloc


# ======================================================================
# Launch 1: build bf16 gather tables (x = cj * feat), row-sharded
# ======================================================================

def build_prep_nc():
    NPC, NT, NPAD, TBL = _derived()
    nc = bass.Bass()
    feat_in = nc.dram_tensor("feat_slice", (2, NPAD, F), f32, kind="ExternalInput")
    # cj host-transposed to [2, 128, NT]: element [s, i, t] = cj[s, t*128+i]
    cj_in = nc.dram_tensor("cj_slice", (2, 128, NT), f32, kind="ExternalInput")
    x_out = nc.dram_tensor("x_slice", (2, NPAD, F), bf16, kind="ExternalOutput")

    with tile.TileContext(nc) as tc:
        with (
            tc.tile_pool(name="cj", bufs=1) as cjp,
            tc.tile_pool(name="sb", bufs=6) as sb,
        ):
            cj_sb = cjp.tile([128, 2 * NT], f32, tag="cj")
            nc.sync.dma_start(
                out=cj_sb[:].rearrange("p (s t) -> p s t", s=2),
                in_=cj_in[:, :, :].rearrange("s p t -> p s t"))
            # absorber: advance DVE's clock past the cj DMA so later consumers
            # need only one wait (walrus allows a single sync wait per compute
            # instruction and the scheduler doesn't always split).
            scratch = cjp.tile([128, 1], f32, tag="scratch")
            nc.vector.tensor_copy(out=scratch[:], in_=cj_sb[:, :1])
            for side in range(2):
                for t in range(NT):
                    rows = slice(t * 128, (t + 1) * 128)
                    ft = sb.tile([128, F], f32, tag="ft")
                    nc.sync.dma_start(out=ft[:], in_=feat_in[side, rows, :])
                    xt = sb.tile([128, F], bf16, tag="xt")
                    c0 = side * NT + t
                    nc.vector.tensor_tensor(
                        out=xt[:], in0=ft[:],
                        in1=cj_sb[:, c0:c0 + 1].to_broadcast([128, F]),
                        op=mybir.AluOpType.mult,
                    )
                    nc.sync.dma_start(out=x_out[side, rows, :], in_=xt[:])
    return nc


# ======================================================================
# Launch 2: main kernel
# ======================================================================

def build_main_nc(C0: int, C1: int, bench: bool = False):
    """bench=True: identical device work, but the big result goes to an
    Internal DRAM tensor and only a tiny tensor is an ExternalOutput --
    keeps PJRT from shipping 16MB/core back per call, so wall-clock
    differencing isolates device exec time."""
    NPC, NT, NPAD, TBL = _derived()
    W1 = TBL - WIN
    C0T, C1T = R * C0, R * C1
    CT = C0T + C1T
    # 4 SWDGE queues (ucode max): round-robined gathers pipeline desc-gen
    # and transfers; measured ~4.1ns/descriptor vs ~7.2ns on one queue.
    nc = bass.Bass(num_swdge_queues=4)
    x_drug = nc.dram_tensor("x_drug", (TBL, F), bf16, kind="ExternalInput")
    x_dis = nc.dram_tensor("x_dis", (TBL, F), bf16, kind="ExternalInput")
    idx0_in = nc.dram_tensor("idx0", (2, 128, NT * C0T * 8), i16, kind="ExternalInput")
    idx1_in = nc.dram_tensor("idx1", (2, 128, NT * C1T * 8), i16, kind="ExternalInput")
    dstloc_in = nc.dram_tensor("dstloc", (2, 128, NT * CT), bf16, kind="ExternalInput")
    # ci host-transposed to [2, 128, NT]
    ci_in = nc.dram_tensor("ci_pad", (2, 128, NT), f32, kind="ExternalInput")
    att_in = nc.dram_tensor("att", (R, NB), f32, kind="ExternalInput")
    basis_in = nc.dram_tensor("basis", (NB, F, F), f32, kind="ExternalInput")
    fcw_in = nc.dram_tensor("fc_w", (F, OUT), f32, kind="ExternalInput")
    fcb_in = nc.dram_tensor("fc_b", (OUT,), f32, kind="ExternalInput")
    if bench:
        out = nc.dram_tensor("out_scratch", (2, NPAD, R, OUT), f32,
                             kind="Internal")
        bench_out = nc.dram_tensor("bench_out", (128, R * OUT), f32,
                                   kind="ExternalOutput")
    else:
        out = nc.dram_tensor("out_part", (2, NPAD, R, OUT), f32,
                             kind="ExternalOutput")

    iota_np = np.broadcast_to(np.arange(128, dtype=np.float32), (128, 128))
    iota_c = nc.inline_tensor(np.ascontiguousarray(iota_np), "iota_c")
    ident_c = nc.inline_tensor(np.eye(128, dtype=np.float32), "ident_c")
    ones_c = nc.inline_tensor(np.ones((1, 128), dtype=np.float32), "ones_c")

    eq = mybir.AluOpType.is_equal
    mult = mybir.AluOpType.mult
    add = mybir.AluOpType.add

    splits0 = _splits(C0T)
    splits1 = _splits(C1T)
    qctr = [0]

    with tile.TileContext(nc) as tc:
        with (
            tc.tile_pool(name="const", bufs=1) as cp,
            tc.tile_pool(name="sb", bufs=4) as sb,
            tc.tile_pool(name="mp", bufs=3) as mp,
            tc.tile_pool(name="pp", bufs=3) as pp,
            tc.tile_pool(name="zp", bufs=4) as zp,
            tc.tile_pool(name="idxp", bufs=1) as idxp,
            tc.tile_pool(name="ps", bufs=4, space="PSUM") as ps,
            tc.tile_pool(name="ps2", bufs=4, space="PSUM") as ps2,
        ):
            nc.gpsimd.load_library(library_config.mlp)
            # one Pool register per distinct gather size (to_reg per call
            # would exhaust the register file across ~700 gathers)
            ni_regs = {gn: nc.gpsimd.to_reg(gn * 128)
                       for (_, gn) in set(splits0) | set(splits1)}
            # ---------- consts ----------
            iota_f = cp.tile([128, 128], f32, tag="iotaf")
            nc.sync.dma_start(out=iota_f[:], in_=iota_c[:, :])
            iota_t = cp.tile([128, 128], bf16, tag="iota")
            nc.vector.tensor_copy(out=iota_t[:], in_=iota_f[:])
            ident_t = cp.tile([128, 128], f32, tag="ident")
            nc.sync.dma_start(out=ident_t[:], in_=ident_c[:, :])
            ones_f32 = cp.tile([1, 128], f32, tag="ones32")
            nc.sync.dma_start(out=ones_f32[:], in_=ones_c[:, :])
            fcw_t = cp.tile([128, OUT], f32, tag="fcw")
            nc.sync.dma_start(out=fcw_t[:], in_=fcw_in[:, :])
            fcb_row = cp.tile([1, OUT], f32, tag="fcbrow")
            nc.sync.dma_start(out=fcb_row[:], in_=fcb_in[None, :])
            att_row = cp.tile([1, R * NB], f32, tag="attrow")
            nc.sync.dma_start(out=att_row[:],
                              in_=att_in[:, :].rearrange("r b -> () (r b)"))

            # ---------- W prep ----------
            attb_ps = ps2.tile([128, R * NB], f32, tag="o2")
            nc.tensor.matmul(out=attb_ps[:], lhsT=ones_f32[:], rhs=att_row[:],
                             start=True, stop=True)
            att_b = cp.tile([128, R * NB], f32, tag="attb")
            nc.vector.tensor_copy(out=att_b[:], in_=attb_ps[:])

            biasb_ps = ps2.tile([128, OUT], f32, tag="o2")
            nc.tensor.matmul(out=biasb_ps[:], lhsT=ones_f32[:], rhs=fcb_row[:],
                             start=True, stop=True)
            bias5 = cp.tile([128, R * OUT], f32, tag="bias5")
            for r in range(R):
                nc.vector.tensor_copy(out=bias5[:, r * OUT:(r + 1) * OUT],
                                      in_=biasb_ps[:])

            # basis[b] transposed: [e, f]
            bT = []
            for b in range(NB):
                bt_in = sb.tile([128, 128], f32, tag="bload")
                nc.sync.dma_start(out=bt_in[:], in_=basis_in[b, :, :])
                bt_ps = ps.tile([128, 128], f32, tag="zt")
                nc.tensor.transpose(out=bt_ps[:], in_=bt_in[:], identity=ident_t[:])
                bt_sb = cp.tile([128, 128], f32, tag=f"bT{b}")
                nc.vector.tensor_copy(out=bt_sb[:], in_=bt_ps[:])
                bT.append(bt_sb)

            wfc = cp.tile([128, R * OUT], bf16, tag="wfc")
            for r in range(R):
                wrt = sb.tile([128, 128], f32, tag="wrt")
                tmp = sb.tile([128, 128], f32, tag="wtmp")
                nc.vector.tensor_tensor(
                    out=tmp[:], in0=bT[1][:],
                    in1=att_b[:, 2 * r + 1:2 * r + 2].to_broadcast([128, 128]),
                    op=mult,
                )
                nc.vector.tensor_tensor(
                    out=wrt[:], in0=bT[0][:],
                    in1=att_b[:, 2 * r:2 * r + 1].to_broadcast([128, 128]),
                    op=mult,
                )
                nc.vector.tensor_tensor(out=wrt[:], in0=wrt[:], in1=tmp[:], op=add)
                wfc_ps = ps2.tile([128, OUT], f32, tag="o2")
                nc.tensor.matmul(out=wfc_ps[:], lhsT=wrt[:], rhs=fcw_t[:],
                                 start=True, stop=True)
                nc.scalar.copy(out=wfc[:, r * OUT:(r + 1) * OUT], in_=wfc_ps[:])

            # ---------- main loops ----------
            ci_sb = cp.tile([128, 2 * NT], f32, tag="ci")
            nc.sync.dma_start(
                out=ci_sb[:].rearrange("p (s t) -> p s t", s=2),
                in_=ci_in[:, :, :].rearrange("s p t -> p s t"))
            for d in range(2):
                x_src = x_dis if d else x_drug
                idx0_t = idxp.tile([128, NT * C0T * 8], i16, tag="idx0")
                nc.sync.dma_start(out=idx0_t[:], in_=idx0_in[d, :, :])
                idx1_t = idxp.tile([128, NT * C1T * 8], i16, tag="idx1")
                nc.sync.dma_start(out=idx1_t[:], in_=idx1_in[d, :, :])
                dl_t = idxp.tile([128, NT * CT], bf16, tag="dl")
                nc.sync.dma_start(out=dl_t[:], in_=dstloc_in[d, :, :])
                for t in range(NT):
                    rows = slice(t * 128, (t + 1) * 128)
                    ci_col = d * NT + t
                    m_t = mp.tile([128, CT * 128], bf16, tag="m")
                    for (go, gn) in splits0:
                        ni = gn * 128
                        nc.gpsimd.dma_gather(
                            m_t[:, go * 128:(go + gn) * 128].rearrange(
                                "p (c w) -> p c w", w=F),
                            x_src[0:WIN, :],
                            idx0_t[:, (t * C0T + go) * 8:(t * C0T + go + gn) * 8],
                            ni, ni_regs[gn], F, queue_num=qctr[0] % 4,
                        )
                        qctr[0] += 1
                    for (go, gn) in splits1:
                        ni = gn * 128
                        nc.gpsimd.dma_gather(
                            m_t[:, (C0T + go) * 128:(C0T + go + gn) * 128]
                            .rearrange("p (c w) -> p c w", w=F),
                            x_src[W1:W1 + WIN, :],
                            idx1_t[:, (t * C1T + go) * 8:(t * C1T + go + gn) * 8],
                            ni, ni_regs[gn], F, queue_num=qctr[0] % 4,
                        )
                        qctr[0] += 1
                    p_t = pp.tile([128, CT * 128], bf16, tag="p")
                    nc.vector.tensor_tensor(
                        out=p_t[:].rearrange("p (c f) -> p c f", c=CT),
                        in0=dl_t[:, t * CT:(t + 1) * CT][:, :, None].to_broadcast(
                            [128, CT, 128]),
                        in1=iota_t[:, None, :].to_broadcast([128, CT, 128]),
                        op=eq,
                    )
                    o2 = ps2.tile([128, R * OUT], f32, tag="o2")
                    for r in range(R):
                        cols = ([r * C0 + j for j in range(C0)] +
                                [C0T + r * C1 + j for j in range(C1)])
                        zt = ps.tile([128, 128], f32, tag="zt")
                        for k, cc in enumerate(cols):
                            nc.tensor.matmul(
                                out=zt[:],
                                lhsT=m_t[:, cc * 128:(cc + 1) * 128],
                                rhs=p_t[:, cc * 128:(cc + 1) * 128],
                                start=(k == 0), stop=(k == len(cols) - 1),
                            )
                        zt_sb = zp.tile([128, 128], bf16, tag="ztsb")
                        nc.scalar.copy(out=zt_sb[:], in_=zt[:])
                        nc.tensor.matmul(
                            out=o2[:, r * OUT:(r + 1) * OUT], lhsT=zt_sb[:],
                            rhs=wfc[:, r * OUT:(r + 1) * OUT],
                            start=True, stop=True,
                        )
                    ob = sb.tile([128, R * OUT], f32, tag="ob")
                    nc.vector.tensor_tensor(
                        out=ob[:], in0=o2[:],
                        in1=ci_sb[:, ci_col:ci_col + 1].to_broadcast(
                            [128, R * OUT]),
                        op=mult,
                    )
                    nc.vector.tensor_tensor(
                        out=ob[:], in0=ob[:], in1=bias5[:], op=add)
                    nc.sync.dma_start(
                        out=out[d, rows, :, :].rearrange("p r o -> p (r o)"),
                        in_=ob[:],
                    )
                    if bench and d == 1 and t == NT - 1:
                        nc.sync.dma_start(out=bench_out[:, :], in_=ob[:])
    mybir.codegen_inst_isa_subclasses(nc)
    return nc


# ======================================================================
# kernel entry
# ======================================================================

_cache: dict = {}


def kernel(drug_feat, dis_feat, cj_drug, ci_drug, cj_dis, ci_dis,
           att, basis, fc_w, fc_b, edge_drug, edge_dis):
    NPC, NT, NPAD, TBL = _derived()
    drug_feat = np.asarray(drug_feat, np.float32)
    dis_feat = np.asarray(dis_feat, np.float32)
    cj_drug = np.asarray(cj_drug, np.float32)
    ci_drug = np.asarray(ci_drug, np.float32)
    cj_dis = np.asarray(cj_dis, np.float32)
    ci_dis = np.asarray(ci_dis, np.float32)
    att = np.asarray(att, np.float32)
    basis = np.asarray(basis, np.float32)
    fc_w = np.asarray(fc_w, np.float32)
    fc_b = np.asarray(fc_b, np.float32)
    edge_drug = np.asarray(edge_drug, np.int32)
    edge_dis = np.asarray(edge_dis, np.int32)

    # ---- host preprocessing: edge sort/shard (index manipulation only) ----
    # direction 0: drug -> dis (dest = dis), direction 1: dis -> drug
    _tlog("start")
    n0a, n1a, ta = _prep_direction(edge_drug, edge_dis, None, None)
    n0b, n1b, tb = _prep_direction(edge_dis, edge_drug, None, None)
    C0 = (max(n0a, n0b) + 127) // 128
    Cm = (max(ta, tb) + 127) // 128
    C1 = max((max(n1a, n1b) + 127) // 128, Cm - C0)
    idx0_d0, idx1_d0, dl_d0 = _prep_direction(edge_drug, edge_dis, C0, C1)
    idx0_d1, idx1_d1, dl_d1 = _prep_direction(edge_dis, edge_drug, C0, C1)
    _tlog(f"host prep done C0={C0} C1={C1}")

    # ---- launch 1: build gather tables ----
    if "prep" not in _cache:
        _cache["prep"] = build_prep_nc()
    nc1 = _cache["prep"]

    in_maps1 = []
    for c in range(NCORES):
        rows = slice(c * NPC, (c + 1) * NPC)
        feat_slice = np.zeros((2, NPAD, F), np.float32)
        feat_slice[0, :NPC] = drug_feat[rows]
        feat_slice[1, :NPC] = dis_feat[rows]
        cj_slice = np.zeros((2, NPAD), np.float32)
        cj_slice[0, :NPC] = cj_drug[rows]
        cj_slice[1, :NPC] = cj_dis[rows]
        cj_slice = np.ascontiguousarray(
            cj_slice.reshape(2, NT, 128).transpose(0, 2, 1))
        in_maps1.append({"feat_slice": feat_slice, "cj_slice": cj_slice})
    _tlog("launch1 inputs built")
    res1 = run_bass_kernel_spmd(nc1, in_maps1, core_ids=list(range(NCORES)))
    _tlog("launch1 done")
    xs = [r["x_slice"] for r in res1.results]
    x_drug_full = np.zeros((TBL, F), BF16)
    x_dis_full = np.zeros((TBL, F), BF16)
    for c in range(NCORES):
        rows = slice(c * NPC, (c + 1) * NPC)
        x_drug_full[rows] = xs[c][0, :NPC]
        x_dis_full[rows] = xs[c][1, :NPC]

    # ---- launch 2: main ----
    global _last_cfg
    _last_cfg = (C0, C1)
    key = ("main", C0, C1)
    if key not in _cache:
        _cache[key] = build_main_nc(C0, C1)
    nc2 = _cache[key]

    in_maps2 = []
    for c in range(NCORES):
        rows = slice(c * NPC, (c + 1) * NPC)
        ci_pad = np.zeros((2, NPAD), np.float32)
        ci_pad[0, :NPC] = ci_dis[rows]    # dir 0 dest = dis
        ci_pad[1, :NPC] = ci_drug[rows]   # dir 1 dest = drug
        ci_pad = np.ascontiguousarray(
            ci_pad.reshape(2, NT, 128).transpose(0, 2, 1))
        in_maps2.append({
            "x_drug": x_drug_full, "x_dis": x_dis_full,
            "idx0": np.stack([idx0_d0[c], idx0_d1[c]], axis=0),
            "idx1": np.stack([idx1_d0[c], idx1_d1[c]], axis=0),
            "dstloc": np.stack([dl_d0[c], dl_d1[c]], axis=0),
            "ci_pad": ci_pad,
            "att": att, "basis": basis, "fc_w": fc_w, "fc_b": fc_b,
        })
    _tlog("launch2 inputs built")
    res2 = run_bass_kernel_spmd(nc2, in_maps2, core_ids=list(range(NCORES)))
    _tlog("launch2 done")

    out_dis = np.concatenate(
        [r["out_part"][0, :NPC] for r in res2.results], axis=0)
    out_drug = np.concatenate(
        [r["out_part"][1, :NPC] for r in res2.results], axis=0)
    _tlog("assembled")
    return out_drug.astype(np.float32), out_dis.astype(np.float32)
